# revision 1
# baseline (speedup 1.0000x reference)
"""BiLSTM-CRF Trainium2 kernel.

Full-input contract: kernel(**inputs) takes the unsharded numpy inputs and
returns the full [64, 512, 32, 32] float32 output. Internally shards the
batch (64) across 8 NeuronCores (8 sentences per core), runs a Bass kernel
SPMD, and concatenates the per-core outputs.

Per-core device pipeline:
  1. Embedding gather (indirect DMA) for fwd token order and reversed order.
  2. PE-transpose gathered rows into xT [E=128, L*B] column-major-by-(t,b).
  3. JIT precompute of P = Wih @ x (+bias) in windows of 32 timesteps.
  4. 512-step LSTM scan for both directions (independent chains, interleaved).
     Gate layout: gates.T [128 partitions = gate-dim-in-chunk, 8 chunks * B].
     Chunk order [i0 i1 f0 f1 o0 o1 g0 g1] so sigmoid(i,f,o) is one ACT op.
  5. Emission matmul emisT[33, t*b] = W_linT_aug.T @ h_cat.T (row 32 == 1).
  6. CRF expand: crf[128 rows=(t,b), 1024=(i,j)] = emisT_slice.T @ Jsel_aug
     where Jsel_aug row 32 carries transition+b_lin -> broadcast add fused
     into the same accumulation. Copy PSUM->SBUF, DMA out.
"""

import numpy as np

VOCAB, EMB, HID, OUT = 30000, 128, 256, 32
B, L = 64, 512
NCORES = 8
BC = B // NCORES  # batch per core = 8

# Scan-path precision: bf16 weights/x/P/h (fp32 PSUM accumulation).
# Halves PE weight-load time (FWL) and enables fast DVE modes.
SCAN_BF16 = True


def _host_prep(inputs, L_eff=L):
    """Prepare per-core in_maps (list of dicts) from full inputs."""
    import concourse.mybir as mybir  # noqa

    sents = np.asarray(inputs["sents_tensor"]).astype(np.int32)  # [B, L]
    emb = np.asarray(inputs["embedding"]).astype(np.float32)  # [V, E]

    # gate permutation: torch order i,f,g,o -> ours i,f,o,g
    perm = np.concatenate([np.arange(0, 512), np.arange(768, 1024),
                           np.arange(512, 768)])

    def mk_wT(Wih, Whh, bih, bhh):
        Wih = np.asarray(Wih, np.float32)[perm]  # [1024, 128]
        Whh = np.asarray(Whh, np.float32)[perm]  # [1024, 256]
        wT = np.stack([
            np.ascontiguousarray(Wih.T),              # [128, 1024]
            np.ascontiguousarray(Whh[:, :128].T),     # [128, 1024]
            np.ascontiguousarray(Whh[:, 128:].T),     # [128, 1024]
        ])  # [3, 128, 1024]
        b = (np.asarray(bih, np.float32) + np.asarray(bhh, np.float32))[perm]
        b_sb = np.ascontiguousarray(b.reshape(8, 128).T)  # [128, 8] col=chunk
        return wT, b_sb

    wT_f, b_f = mk_wT(inputs["Wih_f"], inputs["Whh_f"], inputs["bih_f"],
                      inputs["bhh_f"])
    wT_b, b_b = mk_wT(inputs["Wih_b"], inputs["Whh_b"], inputs["bih_b"],
                      inputs["bhh_b"])
    wT = np.stack([wT_f, wT_b])          # [2, 3, 128, 1024]
    if SCAN_BF16:
        import ml_dtypes
        wT = wT.astype(ml_dtypes.bfloat16)
    bias = np.stack([b_f, b_b], axis=-1)  # [128, 8, 2] col = c + 8*d? keep sep
    bias = np.ascontiguousarray(
        np.concatenate([b_f, b_b], axis=1))  # [128, 16]: cols 0:8 fwd, 8:16 bwd

    W_lin = np.asarray(inputs["W_lin"], np.float32)      # [32, 2H]
    b_lin = np.asarray(inputs["b_lin"], np.float32)      # [32]
    trans = np.asarray(inputs["transition"], np.float32)  # [32, 32]

    WlinT = np.ascontiguousarray(W_lin.T)  # [512, 32]
    WlinT_aug = np.zeros([4, 128, 33], np.float32)
    for kt in range(4):
        WlinT_aug[kt, :, :32] = WlinT[kt * 128:(kt + 1) * 128]
    if SCAN_BF16:
        import ml_dtypes
        WlinT_aug = WlinT_aug.astype(ml_dtypes.bfloat16)

    Jsel_aug = np.zeros([33, 1024], np.float32)
    for i in range(32):
        for j in range(32):
            Jsel_aug[j, i * 32 + j] = 1.0
    Jsel_aug[32] = (trans + b_lin[None, :]).reshape(-1)

    emis_bias = np.zeros([33, 1], np.float32)
    emis_bias[32, 0] = 1.0  # makes emisT row 32 == 1 after ACT copy w/ bias

    in_maps = []
    for c in range(NCORES):
        s = sents[c * BC:(c + 1) * BC, :L_eff]  # [BC, L]
        sT = np.ascontiguousarray(s.T)          # [L, BC] token (t,b)
        sTr = np.ascontiguousarray(s[:, ::-1].T)  # reversed time order
        ntok = L_eff * BC
        ntiles = ntok // 128
        idx = np.ascontiguousarray(
            sT.reshape(ntiles, 128).T).astype(np.int32)   # [128, ntiles]
        idx_r = np.ascontiguousarray(
            sTr.reshape(ntiles, 128).T).astype(np.int32)  # [128, ntiles]
        # pack 4-byte-per-element constants into one [128, *] uint32 tensor:
        # idx (2*ntiles int32) then bias (16 fp32)
        idx_all = np.concatenate([idx, idx_r], axis=1)  # [128, 2*ntiles]
        wT_pm = np.ascontiguousarray(
            wT.transpose(2, 0, 1, 3).reshape(128, -1))     # [128, 6*1024]
        wl_pm = np.ascontiguousarray(
            WlinT_aug.transpose(1, 0, 2).reshape(128, -1))  # [128, 4*33]
        c4_parts = [idx_all.view(np.uint32), bias.view(np.uint32)]
        if SCAN_BF16:
            c2 = np.concatenate([wT_pm, wl_pm], axis=1)  # bf16 lane
        else:
            c2 = None
            c4_parts += [wT_pm.view(np.uint32), wl_pm.view(np.uint32)]
        c4 = np.concatenate(c4_parts, axis=1)
        # jsel + emis ones-row bias column
        cj = np.concatenate([Jsel_aug, emis_bias], axis=1)  # [33, 1025]
        m = {
            "c4": np.ascontiguousarray(c4),
            "emb": emb,
            "cj": np.ascontiguousarray(cj),
        }
        if c2 is not None:
            m["c2"] = np.ascontiguousarray(c2)
        in_maps.append(m)
    return in_maps


def build_nc(L_eff=L, reps=1, timing=False):
    """Build the Bass program (identical for every core).

    reps>1 repeats the compute body (scan+emission) N times; timing=True
    swaps the big external tensors (embedding in, crf out) for internal
    DRAM so benchmark calls transfer almost nothing over the axon tunnel.
    """
    import concourse.bass as bass
    import concourse.mybir as mybir
    import concourse.tile as tile
    from concourse.bacc import Bacc
    from concourse.masks import make_identity

    dt = mybir.dt
    AF = mybir.ActivationFunctionType
    OP = mybir.AluOpType

    NTOK = L_eff * BC          # tokens per direction
    NTILE = NTOK // 128        # gather tiles per direction
    WIN = 32 if L_eff >= 32 else L_eff  # steps per P window
    NWIN = L_eff // WIN
    NW = WIN * BC              # P-window token count (cols per chunk)

    nc = Bacc()

    N4 = 2 * NTILE + 16 + (0 if SCAN_BF16 else 6 * 1024 + 4 * 33)
    d_c4 = nc.declare_dram_parameter("c4", [128, N4], dt.uint32, False)
    if timing:
        d_emb = nc.dram_tensor("embt", [VOCAB, EMB], dt.float32)
    else:
        d_emb = nc.declare_dram_parameter("emb", [VOCAB, EMB], dt.float32,
                                          False)
    d_cj = nc.declare_dram_parameter("cj", [33, 1025], dt.float32r, False)
    if SCAN_BF16:
        d_c2 = nc.declare_dram_parameter("c2", [128, 6 * 1024 + 4 * 33],
                                         dt.bfloat16, False)
    if timing:
        d_out = nc.dram_tensor("outt", [BC, L_eff, 1024], dt.float32)
        d_out_ext = nc.declare_dram_parameter("out", [1, 16], dt.float32,
                                              isOutput=True)
    else:
        d_out = nc.declare_dram_parameter("out", [BC, L_eff, 1024],
                                          dt.float32, isOutput=True)
        d_out_ext = None

    def r(ap):  # fp32 -> fp32r view for fast moving operands
        return ap.bitcast(dt.float32r)

    SDT = dt.bfloat16 if SCAN_BF16 else dt.float32  # scan-path dtype

    def s_r(ap):  # scan-path moving-operand view
        return ap if SCAN_BF16 else ap.bitcast(dt.float32r)

    with tile.TileContext(nc) as tc:
        with (
            tc.tile_pool(name="const", bufs=1) as const,
            tc.tile_pool(name="state", bufs=1) as state,
        ):
            # ---- constants / persistent tiles (3 DMA calls total, to keep
            # per-instruction semaphore-wait counts under the ISA limit) ----
            ident = const.tile([128, 128], dt.float32)
            make_identity(nc, ident[:])
            ident_s = const.tile([128, 128], SDT)
            nc.vector.tensor_copy(out=ident_s[:], in_=ident[:])
            c4_sb = const.tile([128, N4], dt.uint32)
            nc.sync.dma_start(out=c4_sb[:], in_=d_c4[:])
            idx_sb = c4_sb[:, 0:2 * NTILE].bitcast(dt.int32)
            bias_sb = c4_sb[:, 2 * NTILE:2 * NTILE + 16].bitcast(dt.float32)
            if SCAN_BF16:
                c2_sb = const.tile([128, 6 * 1024 + 4 * 33], dt.bfloat16)
                nc.sync.dma_start(out=c2_sb[:], in_=d_c2[:])
                wT_sb = c2_sb[:, 0:6 * 1024]
                wlin_sb = c2_sb[:, 6 * 1024:]
            else:
                o4 = 2 * NTILE + 16
                wT_sb = c4_sb[:, o4:o4 + 6 * 1024].bitcast(dt.float32)
                wlin_sb = c4_sb[:, o4 + 6 * 1024:].bitcast(dt.float32)
            cj_sb = const.tile([33, 1025], dt.float32r)
            nc.sync.dma_start(out=cj_sb[:], in_=d_cj[:])
            jsel_sb = cj_sb[:, 0:1024]
            ebias_sb = cj_sb[:, 1024:1025].bitcast(dt.float32)

            def wTd(d, kt):  # [128, 1024] weight K-tile
                off = (d * 3 + kt) * 1024
                return wT_sb[:, off:off + 1024]

            # persistent big buffers
            xT = state.tile([128, 2 * NTOK], SDT)   # cols: d*NTOK+(t,b)
            h_all = state.tile([128, 2 * L_eff * 16], SDT)
            zero16 = state.tile([128, 16], SDT)
            nc.vector.memset(zero16[:], 0.0)

            # ---- all pools stay open for the whole kernel: releasing a
            # pool lets later pools reuse its SBUF range, which creates
            # WAR deps on every DMA that wrote it (sem-wait-count blowup).
            with (
                tc.tile_pool(name="gat", bufs=4) as gat,
                tc.tile_pool(name="pwin", bufs=2) as pwin,
                tc.tile_pool(name="jit_ps", bufs=2, space="PSUM") as jit_ps,
                tc.tile_pool(name="gates_ps", bufs=3, space="PSUM") as gates_ps,
                tc.tile_pool(name="cpool", bufs=2) as cpool,
                tc.tile_pool(name="spool", bufs=3) as spool,
                tc.tile_pool(name="emis_ps", bufs=1, space="PSUM") as emis_ps,
                tc.tile_pool(name="emis_sb", bufs=2) as emis_sb_p,
                tc.tile_pool(name="crf_ps", bufs=2, space="PSUM") as crf_ps,
                tc.tile_pool(name="crf_sb", bufs=3) as crf_sb_p,
            ):
                # absorb the identity-ready (Pool) wait into a throwaway PE
                # transpose so real transposes carry only their gather wait
                # (matmul ISA sync-wait slots are scarce)
                pt0 = jit_ps.tile([128, NW], dt.float32, tag="jp")
                nc.tensor.transpose(out=pt0[:, 0:128], in_=ident[:],
                                    identity=ident[:])

                # ---- embedding gather + transpose to xT ----
                if timing:
                    # the timing build's embedding table is uninitialized
                    # DRAM; gathered garbage (denormals/NaN) would poison the
                    # whole scan with slow-path arithmetic. Zero xT instead —
                    # gathers are outside the repeated body and don't affect
                    # the marginal-time measurement.
                    nc.vector.memset(xT[:], 0.0)
                for g in range(0 if timing else 2 * NTILE):
                    gt = gat.tile([128, 128], dt.float32, tag="g")
                    nc.gpsimd.indirect_dma_start(
                        out=gt[:], out_offset=None, in_=d_emb[:],
                        in_offset=bass.IndirectOffsetOnAxis(
                            ap=idx_sb[:, g:g + 1], axis=0),
                    )
                    pt = jit_ps.tile([128, NW], dt.float32, tag="jp")
                    nc.tensor.transpose(out=pt[:, 0:128], in_=gt[:],
                                        identity=ident[:])
                    if g % 2 == 0:
                        nc.vector.tensor_copy(
                            out=xT[:, g * 128:(g + 1) * 128], in_=pt[:, 0:128])
                    else:
                        nc.scalar.copy(
                            out=xT[:, g * 128:(g + 1) * 128], in_=pt[:, 0:128])
                def jit_window(d, w):
                    """P window: [128, 8 chunks * NW], col = c*NW + t_l*8 + b."""
                    P = pwin.tile([128, 8 * NW], SDT, tag=f"P{d}")
                    for c in range(8):
                        ps = jit_ps.tile([128, NW], dt.float32, tag="jp")
                        nc.tensor.matmul(
                            out=ps[:],
                            lhsT=s_r(wTd(d, 0)[:, c * 128:(c + 1) * 128]),
                            rhs=s_r(xT[:, d * NTOK + w * NW:
                                       d * NTOK + (w + 1) * NW]),
                            start=True, stop=True)
                        # copy + per-partition bias add
                        if c % 2 == 0:
                            nc.scalar.activation(
                                out=P[:, c * NW:(c + 1) * NW], in_=ps[:],
                                func=AF.Identity,
                                bias=bias_sb[:, d * 8 + c:d * 8 + c + 1])
                        else:
                            nc.vector.tensor_scalar(
                                out=P[:, c * NW:(c + 1) * NW], in0=ps[:],
                                scalar1=bias_sb[:, d * 8 + c:d * 8 + c + 1],
                                scalar2=None, op0=OP.add)
                    return P

                def h_slot(d, t):
                    off = d * L_eff * 16 + t * 16
                    return h_all[:, off:off + 16]

                c_prev = [None, None]
                P_cur = [None, None]

                def scan_step(d, s):
                    """One LSTM step for direction d at step s.
                    fwd: t = s; bwd: t = L-1-s (h written at original t)."""
                    t = s if d == 0 else L_eff - 1 - s
                    w, s_l = divmod(s, WIN)
                    if s_l == 0:
                        P_cur[d] = jit_window(d, w)
                    P = P_cur[d]
                    hp = zero16[:] if s == 0 else h_slot(d, t + (1 if d else -1))
                    g_ps = gates_ps.tile([128, 64], dt.float32, tag="g")
                    # init PSUM with P_t via identity matmul (PE does the add;
                    # measured equal-or-better than a DVE add and shortens the
                    # cross-engine dependency chain)
                    nc.tensor.matmul(
                        out=g_ps[:], lhsT=ident_s[:],
                        rhs=P.rearrange("p (c n) -> p c n", c=8)
                             [:, :, s_l * 8:(s_l + 1) * 8],
                        start=True, stop=True)
                    for c in range(8):
                        for kt in (1, 2):
                            nc.tensor.matmul(
                                out=g_ps[:, c * 8:(c + 1) * 8],
                                lhsT=wTd(d, kt)[:, c * 128:(c + 1) * 128],
                                rhs=hp[:, (kt - 1) * 8:kt * 8],
                                start=False, stop=False,
                                skip_group_check=True)
                    sg = spool.tile([128, 64], dt.float32, tag="s")
                    # sigmoid over i,f,o (cols 0:48); tanh over g (48:64)
                    nc.scalar.activation(out=sg[:, 0:48], in_=g_ps[:, 0:48],
                                         func=AF.Sigmoid)
                    nc.scalar.activation(out=sg[:, 48:64], in_=g_ps[:, 48:64],
                                         func=AF.Tanh)
                    c_new = cpool.tile([128, 16], dt.float32, tag="c")
                    # c_new = sig_i * tanh_g
                    nc.vector.tensor_tensor(out=c_new[:], in0=sg[:, 0:16],
                                            in1=sg[:, 48:64], op=OP.mult)
                    if s > 0:
                        # tmp = sig_f * c_prev  (reuse sg cols 16:32 as scratch)
                        nc.vector.tensor_tensor(out=sg[:, 16:32],
                                                in0=sg[:, 16:32],
                                                in1=c_prev[d][:], op=OP.mult)
                        nc.vector.tensor_tensor(out=c_new[:], in0=c_new[:],
                                                in1=sg[:, 16:32], op=OP.add)
                    c_prev[d] = c_new
                    # tanh(c) -> reuse sg cols 48:64
                    nc.scalar.activation(out=sg[:, 48:64], in_=c_new[:],
                                         func=AF.Tanh)
                    nc.vector.tensor_tensor(out=h_slot(d, t), in0=sg[:, 32:48],
                                            in1=sg[:, 48:64], op=OP.mult)

                def emit_block(k):
                    """Emission + CRF + DMA for t block [k*TB, (k+1)*TB).

                    emisT cols are (b, t)-ordered so each CRF row-tile's 128
                    partitions = (4 b's x TBLK t's) map 1:1 onto a plain
                    d_out[b0:b0+4, t0:t0+TBLK, :] DMA slice.
                    """
                    t0 = k * TBLK
                    n = TBLK * BC  # 256 cols
                    eps = emis_ps.tile([33, n], dt.float32, tag="e")
                    for kt in range(4):
                        d = kt // 2
                        c = kt % 2
                        rhs = h_all.rearrange("p (d t c b) -> p d t c b",
                                              d=2, t=L_eff, c=2)[
                            :, d, t0:t0 + TBLK, c, :].rearrange(
                            "p t b -> p b t")
                        nc.tensor.matmul(
                            out=eps[:],
                            lhsT=s_r(wlin_sb[:, kt * 33:(kt + 1) * 33]),
                            rhs=s_r(rhs), start=(kt == 0), stop=(kt == 3))
                    esb = emis_sb_p.tile([33, n], dt.float32r, tag="e")
                    nc.scalar.activation(out=esb[:], in_=eps[:],
                                         func=AF.Identity, bias=ebias_sb[:])
                    # CRF expand: tiles of 128 rows = 4 b's x TBLK t's
                    for rt in range(n // 128):
                        lhs = esb[:, rt * 128:(rt + 1) * 128]
                        nb = 128 // TBLK  # b's per row-tile
                        for hf in range(2):
                            cps = crf_ps.tile([128, 512], dt.float32, tag="c")
                            nc.tensor.matmul(
                                out=cps[:], lhsT=lhs,
                                rhs=jsel_sb[:, hf * 512:(hf + 1) * 512],
                                start=True, stop=True)
                            csb = crf_sb_p.tile([128, 512], dt.float32, tag="c")
                            if hf == 0:
                                nc.scalar.copy(out=csb[:], in_=cps[:])
                            else:
                                nc.vector.tensor_copy(out=csb[:], in_=cps[:])
                            dst = d_out[rt * nb:(rt + 1) * nb, t0:t0 + TBLK,
                                        hf * 512:(hf + 1) * 512]
                            nc.sync.dma_start(out=dst, in_=csb[:])

                TBLK = 32 if L_eff >= 64 else L_eff  # t's per output block
                NBLK = L_eff // TBLK

                for _rep in range(reps):
                    # middle-out emission schedule: after step s (0-indexed),
                    # t complete iff max(t, L-1-t) <= s.
                    emitted = [False] * NBLK
                    c_prev[0] = c_prev[1] = None
                    P_cur[0] = P_cur[1] = None

                    def maybe_emit(s):
                        for k in range(NBLK):
                            if emitted[k]:
                                continue
                            need = max((k + 1) * TBLK - 1,
                                       L_eff - 1 - k * TBLK)
                            if need <= s:
                                emitted[k] = True
                                emit_block(k)

                    for s in range(L_eff):
                        scan_step(0, s)
                        scan_step(1, s)
                        if s >= (L_eff // 2) and (s % 16 == 15
                                                  or s == L_eff - 1):
                            maybe_emit(s)
                    maybe_emit(L_eff - 1)
                    assert all(emitted)

                if timing:
                    tl = crf_sb_p.tile([1, 16], dt.float32, tag="tl")
                    nc.sync.dma_start(out=tl[:], in_=d_out[0, 0, 0:16])
                    nc.sync.dma_start(out=d_out_ext[:], in_=tl[:])

    nc.finalize()
    return nc


_CACHE = {}


def _get_nc(L_eff=L):
    if L_eff not in _CACHE:
        _CACHE[L_eff] = build_nc(L_eff)
    return _CACHE[L_eff]


def kernel(**inputs):
    from concourse.bass_utils import run_bass_kernel_spmd

    nc = _get_nc(L)
    in_maps = _host_prep(inputs, L)
    res = run_bass_kernel_spmd(nc, in_maps, list(range(NCORES)))
    outs = [res.results[c]["out"].reshape(BC, L, OUT, OUT)
            for c in range(NCORES)]
    return np.concatenate(outs, axis=0)


if __name__ == "__main__":
    nc = build_nc(64)
    print("built OK:", len(nc.m.functions[0].instructions)
          if hasattr(nc.m.functions[0], "instructions") else "?")



# revision 2
# speedup vs baseline: 435.6013x; 435.6013x over previous
"""BiLSTM-CRF Trainium2 kernel (v2: hardware-looped scan, minimal I/O).

Full-input contract: kernel(**inputs) takes the unsharded numpy inputs and
returns the full [64, 512, 32, 32] float32 output. Internally shards the
batch (64) across 8 NeuronCores (8 sentences per core), runs a Bass kernel
SPMD, and assembles the output on host.

Device work per core (all phases inside a For_i rep loop so benchmark
variants repeat the body without growing the NEFF):
  phase 1  For_i over 16 windows: P = Wih @ x (+gate bias) for all 512
           steps, both directions, written to SBUF-resident P_f/P_b
           (bf16, c-major layout [128, c*4096 + t*8 + b]).
  phase 2  For_i over 512 steps (unroll 4): both LSTM directions per
           iteration. Gates [128, 64] PSUM accumulate: identity matmul
           initializes with P_t, 16 small Whh matmuls accumulate the
           recurrent term. Chunk order [i0 i1 f0 f1 o0 o1 g0 g1] so
           sigmoid(i,f,o) is one ACT op. h stored bf16 at slot t+1 (fwd)
           / t (bwd) with zero boundary slots -> no step-0 special case.
  phase 3  emission matmul emisT[32, t*8+b] = W_lin^T-tiles @ h tiles,
           DMA'd straight from PSUM to DRAM [32, 4096] f32.

Host does the embedding gather (-> bf16 xT upload, ~1MB/core instead of a
15MB replicated table) and the CRF broadcast add
out[b,l,i,j] = emis[b,l,j] + transition[i,j] + b_lin[j] (134MB never
crosses the device tunnel; only 0.5MB of emissions per core does).
"""

import numpy as np

VOCAB, EMB, HID, OUT = 30000, 128, 256, 32
B, L = 64, 512
NCORES = 8
BC = B // NCORES  # batch per core = 8


def _host_prep(inputs, L_eff=L):
    """Prepare per-core in_maps (list of dicts) from full inputs."""
    import ml_dtypes

    sents = np.asarray(inputs["sents_tensor"]).astype(np.int32)  # [B, L]
    emb = np.asarray(inputs["embedding"]).astype(np.float32)  # [V, E]

    # gate permutation: torch order i,f,g,o -> ours i,f,o,g
    perm = np.concatenate([np.arange(0, 512), np.arange(768, 1024),
                           np.arange(512, 768)])

    def mk_wT(Wih, Whh, bih, bhh):
        Wih = np.asarray(Wih, np.float32)[perm]  # [1024, 128]
        Whh = np.asarray(Whh, np.float32)[perm]  # [1024, 256]
        wT = np.stack([
            np.ascontiguousarray(Wih.T),              # [128, 1024]
            np.ascontiguousarray(Whh[:, :128].T),     # [128, 1024]
            np.ascontiguousarray(Whh[:, 128:].T),     # [128, 1024]
        ])  # [3, 128, 1024]
        b = (np.asarray(bih, np.float32) + np.asarray(bhh, np.float32))[perm]
        b_sb = np.ascontiguousarray(b.reshape(8, 128).T)  # [128, 8] col=chunk
        return wT, b_sb

    wT_f, b_f = mk_wT(inputs["Wih_f"], inputs["Whh_f"], inputs["bih_f"],
                      inputs["bhh_f"])
    wT_b, b_b = mk_wT(inputs["Wih_b"], inputs["Whh_b"], inputs["bih_b"],
                      inputs["bhh_b"])
    wT = np.stack([wT_f, wT_b]).astype(ml_dtypes.bfloat16)  # [2,3,128,1024]
    bias = np.ascontiguousarray(
        np.concatenate([b_f, b_b], axis=1))  # [128, 16]: cols 0:8 f, 8:16 b

    W_lin = np.asarray(inputs["W_lin"], np.float32)      # [32, 2H]
    WlinT = np.ascontiguousarray(W_lin.T)                # [512, 32]
    wl_pm = np.ascontiguousarray(
        WlinT.reshape(4, 128, 32).transpose(1, 0, 2).reshape(128, 128)
    ).astype(ml_dtypes.bfloat16)                         # [128, 4*32]

    wT_pm = np.ascontiguousarray(
        wT.transpose(2, 0, 1, 3).reshape(128, -1))       # [128, 6*1024]
    c2 = np.concatenate([wT_pm, wl_pm], axis=1)          # [128, 6272] bf16

    # x gather: [B, L, E] fp32 -> per-core xT [128, L*8] bf16, col = t*8+b
    x = emb[sents[:, :L_eff]]  # [B, L_eff, 128] fp32
    in_maps = []
    for c in range(NCORES):
        xc = x[c * BC:(c + 1) * BC]                      # [8, L_eff, 128]
        xT = np.ascontiguousarray(
            xc.transpose(2, 1, 0).reshape(128, L_eff * BC)
        ).astype(ml_dtypes.bfloat16)                     # [128, L*8]
        in_maps.append({
            "c2": np.ascontiguousarray(c2),
            "c4": bias,
            "xt": xT,
        })
    return in_maps


def build_nc(L_eff=L, reps=1, timing=False):
    """Build the Bass program (identical for every core).

    reps>1 repeats the compute body (P-precompute + scan + emission) via
    the outer For_i; the NEFF size does not grow with reps. timing=True
    swaps the big external tensors (xt in, emis out) for internal DRAM so
    benchmark calls transfer almost nothing over the axon tunnel.
    """
    import concourse.bass as bass
    import concourse.mybir as mybir
    import concourse.tile as tile
    from concourse.bacc import Bacc
    from concourse.bass import ds
    from concourse.masks import make_identity

    dt = mybir.dt
    AF = mybir.ActivationFunctionType
    OP = mybir.AluOpType

    NT = L_eff * BC            # tokens per direction (cols of xT)
    WIN = 32 if L_eff >= 32 else L_eff
    NWIN = L_eff // WIN
    NW = WIN * BC              # cols per P window chunk
    U = 4 if L_eff % 4 == 0 else 1   # scan unroll
    EB = 512 if L_eff >= 64 else NT  # emission block cols
    NEB = NT // EB

    nc = Bacc()

    d_c2 = nc.declare_dram_parameter("c2", [128, 6 * 1024 + 4 * 32],
                                     dt.bfloat16, False)
    d_c4 = nc.declare_dram_parameter("c4", [128, 16], dt.float32, False)
    if timing:
        d_xt = None
        d_out = nc.dram_tensor("outt", [32, NT], dt.float32)
        d_out_ext = nc.declare_dram_parameter("out", [1, 16], dt.float32,
                                              isOutput=True)
    else:
        d_xt = nc.declare_dram_parameter("xt", [128, NT], dt.bfloat16, False)
        d_out = nc.declare_dram_parameter("out", [32, NT], dt.float32,
                                          isOutput=True)
        d_out_ext = None

    with tile.TileContext(nc) as tc:
        with (
            tc.tile_pool(name="const", bufs=1) as const,
            tc.tile_pool(name="state", bufs=1) as state,
        ):
            ident = const.tile([128, 128], dt.float32)
            make_identity(nc, ident[:])
            ident_s = const.tile([128, 128], dt.bfloat16)
            nc.vector.tensor_copy(out=ident_s[:], in_=ident[:])
            c2_sb = const.tile([128, 6 * 1024 + 4 * 32], dt.bfloat16)
            nc.sync.dma_start(out=c2_sb[:], in_=d_c2[:])
            wT_sb = c2_sb[:, 0:6 * 1024]
            wlin_sb = c2_sb[:, 6 * 1024:]
            c4_sb = const.tile([128, 16], dt.float32)
            nc.sync.dma_start(out=c4_sb[:], in_=d_c4[:])
            bias_sb = c4_sb

            def wTd(d, kt):  # [128, 1024] weight K-tile
                off = (d * 3 + kt) * 1024
                return wT_sb[:, off:off + 1024]

            xT = state.tile([128, NT], dt.bfloat16)
            if timing:
                nc.vector.memset(xT[:], 0.0)
            else:
                nc.sync.dma_start(out=xT[:], in_=d_xt[:])

            # P layout per dir: [128, c*NT + t*8 + b], bf16
            P_f = state.tile([128, 8 * NT], dt.bfloat16)
            P_b = state.tile([128, 8 * NT], dt.bfloat16)
            # h slots: fwd slot t+1 = h_t (slot 0 = zeros);
            #          bwd slot t   = h_t (slot L = zeros)
            h_f = state.tile([128, (L_eff + 1) * 16], dt.bfloat16)
            h_b = state.tile([128, (L_eff + 1) * 16], dt.bfloat16)
            c_f = state.tile([128, 16], dt.float32)
            c_b = state.tile([128, 16], dt.float32)

            with (
                tc.tile_pool(name="jp", bufs=2, space="PSUM") as jp,
                tc.tile_pool(name="gp", bufs=2, space="PSUM") as gp,
                tc.tile_pool(name="ep", bufs=2, space="PSUM") as ep,
                tc.tile_pool(name="sp", bufs=2) as sp,
                tc.tile_pool(name="mp", bufs=2) as mp,
            ):
                with tc.For_i(0, reps, 1) as _rep:
                    # fresh state per rep
                    nc.vector.memset(h_f[:, 0:16], 0.0)
                    nc.vector.memset(h_b[:, L_eff * 16:(L_eff + 1) * 16], 0.0)
                    nc.vector.memset(c_f[:], 0.0)
                    nc.vector.memset(c_b[:], 0.0)

                    # ---- phase 1: P precompute ----
                    with tc.For_i(0, NWIN, 1) as w:
                        for d in range(2):
                            P_d = P_f if d == 0 else P_b
                            for c in range(8):
                                pp = jp.tile([128, NW], dt.float32, tag="jp")
                                nc.tensor.matmul(
                                    out=pp[:],
                                    lhsT=wTd(d, 0)[:, c * 128:(c + 1) * 128],
                                    rhs=xT[:, ds(w * NW, NW)],
                                    start=True, stop=True)
                                bcol = bias_sb[:, d * 8 + c:d * 8 + c + 1]
                                if c % 2 == 0:
                                    nc.scalar.activation(
                                        out=P_d[:, ds(c * NT + w * NW, NW)],
                                        in_=pp[:], func=AF.Identity,
                                        bias=bcol)
                                else:
                                    nc.vector.tensor_scalar(
                                        out=P_d[:, ds(c * NT + w * NW, NW)],
                                        in0=pp[:], scalar1=bcol,
                                        scalar2=None, op0=OP.add)

                    # ---- phase 2: scan ----
                    def scan_step(d, s):
                        """One LSTM step for direction d at scan index s
                        (a ScalarValue expression). fwd t=s; bwd t=L-1-s."""
                        P_d = P_f if d == 0 else P_b
                        h_d = h_f if d == 0 else h_b
                        c_d = c_f if d == 0 else c_b
                        if d == 0:
                            p_off = s * 8               # t = s
                            hp_off = s * 16             # slot t
                            hw_off = s * 16 + 16        # slot t+1
                        else:
                            p_off = (L_eff - 1) * 8 - s * 8       # t = L-1-s
                            hp_off = L_eff * 16 - s * 16          # slot t+1
                            hw_off = (L_eff - 1) * 16 - s * 16    # slot t
                        g_ps = gp.tile([128, 64], dt.float32, tag="g")
                        nc.tensor.matmul(
                            out=g_ps[:], lhsT=ident_s[:],
                            rhs=P_d.rearrange("p (c n) -> p c n", c=8)
                                 [:, :, ds(p_off, 8)],
                            start=True, stop=True)
                        for c in range(8):
                            for kt in (1, 2):
                                nc.tensor.matmul(
                                    out=g_ps[:, c * 8:(c + 1) * 8],
                                    lhsT=wTd(d, kt)[:, c * 128:(c + 1) * 128],
                                    rhs=h_d[:, ds(hp_off + (kt - 1) * 8, 8)],
                                    start=False, stop=False,
                                    skip_group_check=True)
                        sg = sp.tile([128, 64], dt.float32, tag="s")
                        nc.scalar.activation(out=sg[:, 0:48],
                                             in_=g_ps[:, 0:48],
                                             func=AF.Sigmoid)
                        nc.scalar.activation(out=sg[:, 48:64],
                                             in_=g_ps[:, 48:64],
                                             func=AF.Tanh)
                        # f*c_prev -> scratch (must read c_d before overwrite)
                        nc.vector.tensor_tensor(out=sg[:, 16:32],
                                                in0=sg[:, 16:32],
                                                in1=c_d[:], op=OP.mult)
                        nc.vector.tensor_tensor(out=c_d[:], in0=sg[:, 0:16],
                                                in1=sg[:, 48:64], op=OP.mult)
                        nc.vector.tensor_tensor(out=c_d[:], in0=c_d[:],
                                                in1=sg[:, 16:32], op=OP.add)
                        nc.scalar.activation(out=sg[:, 48:64], in_=c_d[:],
                                             func=AF.Tanh)
                        nc.vector.tensor_tensor(out=h_d[:, ds(hw_off, 16)],
                                                in0=sg[:, 32:48],
                                                in1=sg[:, 48:64], op=OP.mult)

                    with tc.For_i(0, L_eff, U) as s0:
                        for u in range(U):
                            scan_step(0, s0 + u)
                            scan_step(1, s0 + u)

                    # ---- phase 3: emission ----
                    for blk in range(NEB):
                        t0 = blk * (EB // 8)
                        nt = EB // 8
                        eps = ep.tile([32, EB], dt.float32, tag="e")
                        for kt in range(4):
                            h_d = h_f if kt < 2 else h_b
                            c = kt % 2
                            off = 1 if kt < 2 else 0  # fwd slot t+1, bwd t
                            rhs = h_d.rearrange("p (t x) -> p t x", x=16)[
                                :, t0 + off:t0 + off + nt, c * 8:(c + 1) * 8]
                            nc.tensor.matmul(
                                out=eps[:],
                                lhsT=wlin_sb[:, kt * 32:(kt + 1) * 32],
                                rhs=rhs, start=(kt == 0), stop=(kt == 3))
                        esb = mp.tile([32, EB], dt.float32, tag="m")
                        if blk % 2 == 0:
                            nc.scalar.copy(out=esb[:], in_=eps[:])
                        else:
                            nc.vector.tensor_copy(out=esb[:], in_=eps[:])
                        nc.sync.dma_start(
                            out=d_out[:, blk * EB:(blk + 1) * EB],
                            in_=esb[:])

                if timing:
                    tl = mp.tile([1, 16], dt.float32, tag="tl")
                    nc.sync.dma_start(out=tl[:], in_=d_out[0:1, 0:16])
                    nc.sync.dma_start(out=d_out_ext[:], in_=tl[:])

    nc.finalize()
    return nc


_CACHE = {}


def _get_nc(L_eff=L, reps=1, timing=False):
    key = (L_eff, reps, timing)
    if key not in _CACHE:
        _CACHE[key] = build_nc(L_eff, reps, timing)
    return _CACHE[key]


def _assemble(outs, inputs, L_eff=L):
    """Host CRF broadcast: out[b,l,i,j] = emis[b,l,j] + trans[i,j] + b_lin[j]."""
    import concurrent.futures as cf

    M = (np.asarray(inputs["transition"], np.float32)
         + np.asarray(inputs["b_lin"], np.float32)[None, :])  # [32, 32]
    res = np.empty((B, L_eff, OUT, OUT), np.float32)

    def fill(c):
        emis = outs[c].reshape(32, L_eff, BC).transpose(2, 1, 0)  # [8, L, 32]
        np.add(emis[:, :, None, :], M[None, None, :, :],
               out=res[c * BC:(c + 1) * BC])

    with cf.ThreadPoolExecutor(max_workers=8) as ex:
        list(ex.map(fill, range(NCORES)))
    return res


def kernel(**inputs):
    from concourse.bass_utils import run_bass_kernel_spmd

    nc = _get_nc(L, 1, False)
    in_maps = _host_prep(inputs, L)
    res = run_bass_kernel_spmd(nc, in_maps, list(range(NCORES)))
    outs = [res.results[c]["out"] for c in range(NCORES)]
    return _assemble(outs, inputs, L)


if __name__ == "__main__":
    nc = build_nc(64)
    print("built OK")


# revision 13
# speedup vs baseline: 1327.8591x; 3.0483x over previous
"""BiLSTM-CRF Trainium2 kernel (v2: hardware-looped scan, minimal I/O).

Full-input contract: kernel(**inputs) takes the unsharded numpy inputs and
returns the full [64, 512, 32, 32] float32 output. Internally shards the
batch (64) across 8 NeuronCores (8 sentences per core), runs a Bass kernel
SPMD, and assembles the output on host.

Device work per core (all phases inside a For_i rep loop so benchmark
variants repeat the body without growing the NEFF):
  phase 1  For_i over 16 windows: P = Wih @ x (+gate bias) for all 512
           steps, both directions, written to SBUF-resident P_f/P_b
           (bf16, c-major layout [128, c*4096 + t*8 + b]).
  phase 2  For_i over 512 steps (unroll 4): both LSTM directions per
           iteration. Gates [128, 64] PSUM accumulate: identity matmul
           initializes with P_t, 16 small Whh matmuls accumulate the
           recurrent term. Chunk order [i0 i1 f0 f1 o0 o1 g0 g1] so
           sigmoid(i,f,o) is one ACT op. h stored bf16 at slot t+1 (fwd)
           / t (bwd) with zero boundary slots -> no step-0 special case.
  phase 3  emission matmul emisT[32, t*8+b] = W_lin^T-tiles @ h tiles,
           DMA'd straight from PSUM to DRAM [32, 4096] f32.

Host does the embedding gather (-> bf16 xT upload, ~1MB/core instead of a
15MB replicated table) and the CRF broadcast add
out[b,l,i,j] = emis[b,l,j] + transition[i,j] + b_lin[j] (134MB never
crosses the device tunnel; only 0.5MB of emissions per core does).
"""

import numpy as np

VOCAB, EMB, HID, OUT = 30000, 128, 256, 32
B, L = 64, 512
NCORES = 8
BC = B // NCORES  # batch per core = 8


def _host_prep(inputs, L_eff=L):
    """Prepare per-core in_maps (list of dicts) from full inputs."""
    import ml_dtypes

    sents = np.asarray(inputs["sents_tensor"]).astype(np.int32)  # [B, L]
    emb = np.asarray(inputs["embedding"]).astype(np.float32)  # [V, E]

    # gate permutation: torch order i,f,g,o -> ours i,f,o,g
    perm = np.concatenate([np.arange(0, 512), np.arange(768, 1024),
                           np.arange(512, 768)])

    def mk_wT(Wih, Whh, bih, bhh):
        Wih = np.asarray(Wih, np.float32)[perm]  # [1024, 128]
        Whh = np.asarray(Whh, np.float32)[perm]  # [1024, 256]
        wT = np.stack([
            np.ascontiguousarray(Wih.T),              # [128, 1024]
            np.ascontiguousarray(Whh[:, :128].T),     # [128, 1024]
            np.ascontiguousarray(Whh[:, 128:].T),     # [128, 1024]
        ])  # [3, 128, 1024]
        b = (np.asarray(bih, np.float32) + np.asarray(bhh, np.float32))[perm]
        b_sb = np.ascontiguousarray(b.reshape(8, 128).T)  # [128, 8] col=chunk
        return wT, b_sb

    wT_f, b_f = mk_wT(inputs["Wih_f"], inputs["Whh_f"], inputs["bih_f"],
                      inputs["bhh_f"])
    wT_b, b_b = mk_wT(inputs["Wih_b"], inputs["Whh_b"], inputs["bih_b"],
                      inputs["bhh_b"])
    wT = np.stack([wT_f, wT_b]).astype(ml_dtypes.bfloat16)  # [2,3,128,1024]
    bias = np.ascontiguousarray(
        np.concatenate([b_f, b_b], axis=1))  # [128, 16]: cols 0:8 f, 8:16 b

    W_lin = np.asarray(inputs["W_lin"], np.float32)      # [32, 2H]
    WlinT = np.ascontiguousarray(W_lin.T)                # [512, 32]
    wl_pm = np.ascontiguousarray(
        WlinT.reshape(4, 128, 32).transpose(1, 0, 2).reshape(128, 128)
    ).astype(ml_dtypes.bfloat16)                         # [128, 4*32]

    wT_pm = np.ascontiguousarray(
        wT.transpose(2, 0, 1, 3).reshape(128, -1))       # [128, 6*1024]
    c2 = np.concatenate([wT_pm, wl_pm], axis=1)          # [128, 6272] bf16

    # x gather: [B, L, E] fp32 -> per-core xT [128, L*8] bf16, col = t*8+b
    x = emb[sents[:, :L_eff]]  # [B, L_eff, 128] fp32
    in_maps = []
    for c in range(NCORES):
        xc = x[c * BC:(c + 1) * BC]                      # [8, L_eff, 128]
        xT = np.ascontiguousarray(
            xc.transpose(2, 1, 0).reshape(128, L_eff * BC)
        ).astype(ml_dtypes.bfloat16)                     # [128, L*8]
        in_maps.append({
            "c2": np.ascontiguousarray(c2),
            "c4": bias,
            "xt": xT,
        })
    return in_maps


def build_nc(L_eff=L, reps=1, timing=False, U=4, staggered=False,
             hints=(), skip_scan=False, skip_rest=False):
    """Build the Bass program (identical for every core).

    reps>1 repeats the compute body (P-precompute + scan + emission) via
    the outer For_i; the NEFF size does not grow with reps. timing=True
    swaps the big external tensors (xt in, emis out) for internal DRAM so
    benchmark calls transfer almost nothing over the axon tunnel.
    """
    import concourse.bass as bass
    import concourse.mybir as mybir
    import concourse.tile as tile
    from concourse.bacc import Bacc
    from concourse.bass import ds
    from concourse.masks import make_identity

    dt = mybir.dt
    AF = mybir.ActivationFunctionType
    OP = mybir.AluOpType

    NT = L_eff * BC            # tokens per direction (cols of xT)
    WIN = 32 if L_eff >= 32 else L_eff
    NWIN = L_eff // WIN
    NW = WIN * BC              # cols per P window chunk
    if L_eff % U != 0:
        U = 1                  # scan unroll
    EB = 512 if L_eff >= 64 else NT  # emission block cols
    NEB = NT // EB

    nc = Bacc()

    d_c2 = nc.declare_dram_parameter("c2", [128, 6 * 1024 + 4 * 32],
                                     dt.bfloat16, False)
    d_c4 = nc.declare_dram_parameter("c4", [128, 16], dt.float32, False)
    if timing:
        d_xt = None
        d_out = nc.dram_tensor("outt", [32, NT], dt.float32)
        d_out_ext = nc.declare_dram_parameter("out", [1, 16], dt.float32,
                                              isOutput=True)
    else:
        d_xt = nc.declare_dram_parameter("xt", [128, NT], dt.bfloat16, False)
        d_out = nc.declare_dram_parameter("out", [32, NT], dt.float32,
                                          isOutput=True)
        d_out_ext = None

    with tile.TileContext(nc) as tc:
        with (
            tc.tile_pool(name="const", bufs=1) as const,
            tc.tile_pool(name="state", bufs=1) as state,
        ):
            ident = const.tile([128, 128], dt.float32)
            make_identity(nc, ident[:])
            ident_s = const.tile([128, 128], dt.bfloat16)
            nc.vector.tensor_copy(out=ident_s[:], in_=ident[:])
            c2_sb = const.tile([128, 6 * 1024 + 4 * 32], dt.bfloat16)
            nc.sync.dma_start(out=c2_sb[:], in_=d_c2[:])
            wT_sb = c2_sb[:, 0:6 * 1024]
            wlin_sb = c2_sb[:, 6 * 1024:]
            c4_sb = const.tile([128, 16], dt.float32)
            nc.sync.dma_start(out=c4_sb[:], in_=d_c4[:])
            bias_sb = c4_sb

            def wTd(d, kt):  # [128, 1024] weight K-tile
                off = (d * 3 + kt) * 1024
                return wT_sb[:, off:off + 1024]

            xT = state.tile([128, NT], dt.bfloat16)
            if timing:
                nc.vector.memset(xT[:], 0.0)
            else:
                nc.sync.dma_start(out=xT[:], in_=d_xt[:])

            # P layout per dir: [128, c*NT + t*8 + b], bf16
            P_f = state.tile([128, 8 * NT], dt.bfloat16)
            P_b = state.tile([128, 8 * NT], dt.bfloat16)
            # h slots: fwd slot t+1 = h_t (slot 0 = zeros);
            #          bwd slot t   = h_t (slot L = zeros)
            h_f = state.tile([128, (L_eff + 1) * 16], dt.bfloat16)
            h_b = state.tile([128, (L_eff + 1) * 16], dt.bfloat16)
            c_f = state.tile([128, 16], dt.float32)
            c_b = state.tile([128, 16], dt.float32)
            # static ping-pong h tiles for the recurrence: the 16 Whh
            # matmuls/step read these at static addresses (a dynamic rhs
            # costs a FusedRegOps per matmul on the PE queue); the
            # t-indexed h history for emission is written by the Pool
            # engine off the critical path.
            h_pp = [[state.tile([128, 16], dt.bfloat16, name=f"hpp{d}{k}")
                     for k in range(2)] for d in range(2)]

            with (
                tc.tile_pool(name="jp", bufs=2, space="PSUM") as jp,
                tc.tile_pool(name="gp", bufs=4, space="PSUM") as gp,
                tc.tile_pool(name="ep", bufs=2, space="PSUM") as ep,
                tc.tile_pool(name="sp", bufs=4) as sp,
                tc.tile_pool(name="mp", bufs=2) as mp,
            ):
                with tc.For_i(0, reps, 1) as _rep:
                    # fresh state per rep
                    nc.vector.memset(h_pp[0][0][:], 0.0)
                    nc.vector.memset(h_pp[1][0][:], 0.0)
                    nc.vector.memset(c_f[:], 0.0)
                    nc.vector.memset(c_b[:], 0.0)

                    # ---- phase 1: P precompute ----
                    with tc.For_i(0, 0 if skip_rest else NWIN, 1) as w:
                        for d in range(2):
                            P_d = P_f if d == 0 else P_b
                            for c in range(8):
                                pp = jp.tile([128, NW], dt.float32, tag="jp")
                                nc.tensor.matmul(
                                    out=pp[:],
                                    lhsT=wTd(d, 0)[:, c * 128:(c + 1) * 128],
                                    rhs=xT[:, ds(w * NW, NW)],
                                    start=True, stop=True)
                                bcol = bias_sb[:, d * 8 + c:d * 8 + c + 1]
                                if c % 2 == 0:
                                    nc.scalar.activation(
                                        out=P_d[:, ds(c * NT + w * NW, NW)],
                                        in_=pp[:], func=AF.Identity,
                                        bias=bcol)
                                else:
                                    nc.vector.tensor_scalar(
                                        out=P_d[:, ds(c * NT + w * NW, NW)],
                                        in0=pp[:], scalar1=bcol,
                                        scalar2=None, op0=OP.add)

                    # ---- phase 2: scan ----
                    def scan_step(d, s, u):
                        """One LSTM step for direction d at scan index s
                        (a ScalarValue expression). fwd t=s; bwd t=L-1-s."""
                        P_d = P_f if d == 0 else P_b
                        h_d = h_f if d == 0 else h_b
                        c_d = c_f if d == 0 else c_b
                        if d == 0:
                            p_off = s * 8               # t = s
                            hw_off = s * 16 + 16        # slot t+1
                        else:
                            p_off = (L_eff - 1) * 8 - s * 8       # t = L-1-s
                            hw_off = (L_eff - 1) * 16 - s * 16    # slot t
                        hp = h_pp[d][u % 2]
                        hw = h_pp[d][(u + 1) % 2]
                        g_ps = gp.tile([128, 64], dt.float32, tag="g")
                        nc.tensor.matmul(
                            out=g_ps[:], lhsT=ident_s[:],
                            rhs=P_d.rearrange("p (c n) -> p c n", c=8)
                                 [:, :, ds(p_off, 8)],
                            start=True, stop=True)
                        for c in range(8):
                            for kt in (1, 2):
                                nc.tensor.matmul(
                                    out=g_ps[:, c * 8:(c + 1) * 8],
                                    lhsT=wTd(d, kt)[:, c * 128:(c + 1) * 128],
                                    rhs=hp[:, (kt - 1) * 8:kt * 8],
                                    start=False, stop=False,
                                    skip_group_check=True)
                        sg = sp.tile([128, 64], dt.float32, tag="s")
                        nc.scalar.activation(out=sg[:, 0:48],
                                             in_=g_ps[:, 0:48],
                                             func=AF.Sigmoid)
                        nc.scalar.activation(out=sg[:, 48:64],
                                             in_=g_ps[:, 48:64],
                                             func=AF.Tanh)
                        # f*c_prev -> scratch (must read c_d before overwrite)
                        nc.vector.tensor_tensor(out=sg[:, 16:32],
                                                in0=sg[:, 16:32],
                                                in1=c_d[:], op=OP.mult)
                        nc.vector.tensor_tensor(out=c_d[:], in0=sg[:, 0:16],
                                                in1=sg[:, 48:64], op=OP.mult)
                        nc.vector.tensor_tensor(out=c_d[:], in0=c_d[:],
                                                in1=sg[:, 16:32], op=OP.add)
                        nc.scalar.activation(out=sg[:, 48:64], in_=c_d[:],
                                             func=AF.Tanh)
                        nc.vector.tensor_tensor(out=hw[:],
                                                in0=sg[:, 32:48],
                                                in1=sg[:, 48:64], op=OP.mult)
                        # t-indexed history for emission: duplicate the
                        # multiply on the (idle) Pool engine, off the
                        # recurrence critical path.
                        nc.gpsimd.tensor_tensor(out=h_d[:, ds(hw_off, 16)],
                                                in0=sg[:, 32:48],
                                                in1=sg[:, 48:64], op=OP.mult)

                    with tc.For_i(0, 0 if skip_scan else L_eff, U,
                                  staggered_reset=staggered,
                                  hint_engines=hints) as s0:
                        for u in range(U):
                            scan_step(0, s0 + u, u)
                            scan_step(1, s0 + u, u)

                    # ---- phase 3: emission ----
                    for blk in range(0 if skip_rest else NEB):
                        t0 = blk * (EB // 8)
                        nt = EB // 8
                        eps = ep.tile([32, EB], dt.float32, tag="e")
                        for kt in range(4):
                            h_d = h_f if kt < 2 else h_b
                            c = kt % 2
                            off = 1 if kt < 2 else 0  # fwd slot t+1, bwd t
                            rhs = h_d.rearrange("p (t x) -> p t x", x=16)[
                                :, t0 + off:t0 + off + nt, c * 8:(c + 1) * 8]
                            nc.tensor.matmul(
                                out=eps[:],
                                lhsT=wlin_sb[:, kt * 32:(kt + 1) * 32],
                                rhs=rhs, start=(kt == 0), stop=(kt == 3))
                        esb = mp.tile([32, EB], dt.float32, tag="m")
                        if blk % 2 == 0:
                            nc.scalar.copy(out=esb[:], in_=eps[:])
                        else:
                            nc.vector.tensor_copy(out=esb[:], in_=eps[:])
                        nc.sync.dma_start(
                            out=d_out[:, blk * EB:(blk + 1) * EB],
                            in_=esb[:])

                if timing:
                    tl = mp.tile([1, 16], dt.float32, tag="tl")
                    nc.sync.dma_start(out=tl[:], in_=d_out[0:1, 0:16])
                    nc.sync.dma_start(out=d_out_ext[:], in_=tl[:])

    nc.finalize()
    return nc


_CACHE = {}


def _get_nc(L_eff=L, reps=1, timing=False):
    key = (L_eff, reps, timing)
    if key not in _CACHE:
        _CACHE[key] = build_nc(L_eff, reps, timing)
    return _CACHE[key]


def _assemble(outs, inputs, L_eff=L):
    """Host CRF broadcast: out[b,l,i,j] = emis[b,l,j] + trans[i,j] + b_lin[j]."""
    import concurrent.futures as cf

    M = (np.asarray(inputs["transition"], np.float32)
         + np.asarray(inputs["b_lin"], np.float32)[None, :])  # [32, 32]
    res = np.empty((B, L_eff, OUT, OUT), np.float32)

    def fill(c):
        emis = outs[c].reshape(32, L_eff, BC).transpose(2, 1, 0)  # [8, L, 32]
        np.add(emis[:, :, None, :], M[None, None, :, :],
               out=res[c * BC:(c + 1) * BC])

    with cf.ThreadPoolExecutor(max_workers=8) as ex:
        list(ex.map(fill, range(NCORES)))
    return res


def kernel(**inputs):
    from concourse.bass_utils import run_bass_kernel_spmd

    nc = _get_nc(L, 1, False)
    in_maps = _host_prep(inputs, L)
    res = run_bass_kernel_spmd(nc, in_maps, list(range(NCORES)))
    outs = [res.results[c]["out"] for c in range(NCORES)]
    return _assemble(outs, inputs, L)


if __name__ == "__main__":
    nc = build_nc(64)
    print("built OK")


# revision 46
# speedup vs baseline: 1372.5124x; 1.0336x over previous
"""BiLSTM-CRF Trainium2 kernel (v2: hardware-looped scan, minimal I/O).

Full-input contract: kernel(**inputs) takes the unsharded numpy inputs and
returns the full [64, 512, 32, 32] float32 output. Internally shards the
batch (64) across 8 NeuronCores (8 sentences per core), runs a Bass kernel
SPMD, and assembles the output on host.

Device work per core (all phases inside a For_i rep loop so benchmark
variants repeat the body without growing the NEFF):
  phase 1  For_i over 16 windows: P = Wih @ x (+gate bias) for all 512
           steps, both directions, written to SBUF-resident P_f/P_b
           (bf16, c-major layout [128, c*4096 + t*8 + b]).
  phase 2  For_i over 512 steps (unroll 4): both LSTM directions per
           iteration. Gates [128, 64] PSUM accumulate: identity matmul
           initializes with P_t, 16 small Whh matmuls accumulate the
           recurrent term. Chunk order [i0 i1 f0 f1 o0 o1 g0 g1] so
           sigmoid(i,f,o) is one ACT op. h stored bf16 at slot t+1 (fwd)
           / t (bwd) with zero boundary slots -> no step-0 special case.
  phase 3  emission matmul emisT[32, t*8+b] = W_lin^T-tiles @ h tiles,
           DMA'd straight from PSUM to DRAM [32, 4096] f32.

Host does the embedding gather (-> bf16 xT upload, ~1MB/core instead of a
15MB replicated table) and the CRF broadcast add
out[b,l,i,j] = emis[b,l,j] + transition[i,j] + b_lin[j] (134MB never
crosses the device tunnel; only 0.5MB of emissions per core does).
"""

import numpy as np

VOCAB, EMB, HID, OUT = 30000, 128, 256, 32
B, L = 64, 512
NCORES = 8
BC = B // NCORES  # batch per core = 8


def _host_prep(inputs, L_eff=L):
    """Prepare per-core in_maps (list of dicts) from full inputs."""
    import ml_dtypes

    sents = np.asarray(inputs["sents_tensor"]).astype(np.int32)  # [B, L]
    emb = np.asarray(inputs["embedding"]).astype(np.float32)  # [V, E]

    # gate permutation: torch order i,f,g,o -> ours i,f,o,g
    perm = np.concatenate([np.arange(0, 512), np.arange(768, 1024),
                           np.arange(512, 768)])

    def mk_wT(Wih, Whh, bih, bhh):
        Wih = np.asarray(Wih, np.float32)[perm]  # [1024, 128]
        Whh = np.asarray(Whh, np.float32)[perm]  # [1024, 256]
        wT = np.stack([
            np.ascontiguousarray(Wih.T),              # [128, 1024]
            np.ascontiguousarray(Whh[:, :128].T),     # [128, 1024]
            np.ascontiguousarray(Whh[:, 128:].T),     # [128, 1024]
        ])  # [3, 128, 1024]
        b = (np.asarray(bih, np.float32) + np.asarray(bhh, np.float32))[perm]
        b_sb = np.ascontiguousarray(b.reshape(8, 128).T)  # [128, 8] col=chunk
        return wT, b_sb

    wT_f, b_f = mk_wT(inputs["Wih_f"], inputs["Whh_f"], inputs["bih_f"],
                      inputs["bhh_f"])
    wT_b, b_b = mk_wT(inputs["Wih_b"], inputs["Whh_b"], inputs["bih_b"],
                      inputs["bhh_b"])
    wT = np.stack([wT_f, wT_b]).astype(ml_dtypes.bfloat16)  # [2,3,128,1024]
    bias = np.ascontiguousarray(
        np.concatenate([b_f, b_b], axis=1))  # [128, 16]: cols 0:8 f, 8:16 b

    W_lin = np.asarray(inputs["W_lin"], np.float32)      # [32, 2H]
    WlinT = np.ascontiguousarray(W_lin.T)                # [512, 32]
    wl_pm = np.ascontiguousarray(
        WlinT.reshape(4, 128, 32).transpose(1, 0, 2).reshape(128, 128)
    ).astype(ml_dtypes.bfloat16)                         # [128, 4*32]

    wT_pm = np.ascontiguousarray(
        wT.transpose(2, 0, 1, 3).reshape(128, -1))       # [128, 6*1024]
    c2 = np.concatenate([wT_pm, wl_pm], axis=1)          # [128, 6272] bf16

    # x gather: [B, L, E] fp32 -> per-core xT [128, L*8] bf16, col = t*8+b
    x = emb[sents[:, :L_eff]]  # [B, L_eff, 128] fp32
    in_maps = []
    for c in range(NCORES):
        xc = x[c * BC:(c + 1) * BC]                      # [8, L_eff, 128]
        xT = np.ascontiguousarray(
            xc.transpose(2, 1, 0).reshape(128, L_eff * BC)
        ).astype(ml_dtypes.bfloat16)                     # [128, L*8]
        in_maps.append({
            "c2": np.ascontiguousarray(c2),
            "c4": bias,
            "xt": xT,
        })
    return in_maps


def build_nc(L_eff=L, reps=1, timing=False, U=4, staggered=False,
             hints=(), skip_scan=False, skip_rest=False):
    """Build the Bass program (identical for every core).

    reps>1 repeats the compute body (P-precompute + scan + emission) via
    the outer For_i; the NEFF size does not grow with reps. timing=True
    swaps the big external tensors (xt in, emis out) for internal DRAM so
    benchmark calls transfer almost nothing over the axon tunnel.
    """
    import concourse.bass as bass
    import concourse.mybir as mybir
    import concourse.tile as tile
    from concourse.bacc import Bacc
    from concourse.bass import ds
    from concourse.masks import make_identity

    dt = mybir.dt
    AF = mybir.ActivationFunctionType
    OP = mybir.AluOpType

    NT = L_eff * BC            # tokens per direction (cols of xT)
    WIN = 32 if L_eff >= 32 else L_eff
    NWIN = L_eff // WIN
    NW = WIN * BC              # cols per P window chunk
    if L_eff % U != 0:
        U = 1                  # scan unroll
    EB = 512 if L_eff >= 64 else NT  # emission block cols
    NEB = NT // EB

    nc = Bacc()

    d_c2 = nc.declare_dram_parameter("c2", [128, 6 * 1024 + 4 * 32],
                                     dt.bfloat16, False)
    d_c4 = nc.declare_dram_parameter("c4", [128, 16], dt.float32, False)
    if timing:
        d_xt = None
        d_out = nc.dram_tensor("outt", [32, NT], dt.float32)
        d_out_ext = nc.declare_dram_parameter("out", [1, 16], dt.float32,
                                              isOutput=True)
    else:
        d_xt = nc.declare_dram_parameter("xt", [128, NT], dt.bfloat16, False)
        d_out = nc.declare_dram_parameter("out", [32, NT], dt.float32,
                                          isOutput=True)
        d_out_ext = None

    with tile.TileContext(nc) as tc:
        with (
            tc.tile_pool(name="const", bufs=1) as const,
            tc.tile_pool(name="state", bufs=1) as state,
        ):
            ident = const.tile([128, 128], dt.float32)
            make_identity(nc, ident[:])
            ident_s = const.tile([128, 128], dt.bfloat16)
            nc.vector.tensor_copy(out=ident_s[:], in_=ident[:])
            c2_sb = const.tile([128, 6 * 1024 + 4 * 32], dt.bfloat16)
            nc.sync.dma_start(out=c2_sb[:], in_=d_c2[:])
            wT_sb = c2_sb[:, 0:6 * 1024]
            wlin_sb = c2_sb[:, 6 * 1024:]
            c4_sb = const.tile([128, 16], dt.float32)
            nc.sync.dma_start(out=c4_sb[:], in_=d_c4[:])
            bias_sb = c4_sb
            # dummy activation so the sigmoid/tanh table set is resident on
            # every path into the scan loop -> the table-load pass can keep
            # InstLoadActFuncSet out of the loop body
            warm = const.tile([128, 1], dt.float32)
            nc.scalar.activation(out=warm[:], in_=ident[:, 0:1],
                                 func=AF.Sigmoid)

            def wTd(d, kt):  # [128, 1024] weight K-tile
                off = (d * 3 + kt) * 1024
                return wT_sb[:, off:off + 1024]

            xT = state.tile([128, NT], dt.bfloat16)
            if timing:
                nc.vector.memset(xT[:], 0.0)
            else:
                nc.sync.dma_start(out=xT[:], in_=d_xt[:])

            # P layout per dir: [128, c*NT + t*8 + b], bf16
            P_f = state.tile([128, 8 * NT], dt.bfloat16)
            P_b = state.tile([128, 8 * NT], dt.bfloat16)
            # h slots: fwd slot t+1 = h_t (slot 0 = zeros);
            #          bwd slot t   = h_t (slot L = zeros)
            h_f = state.tile([128, (L_eff + 1) * 16], dt.bfloat16)
            h_b = state.tile([128, (L_eff + 1) * 16], dt.bfloat16)
            c_f = state.tile([128, 16], dt.float32)
            c_b = state.tile([128, 16], dt.float32)
            # static ping-pong h tiles for the recurrence: the 16 Whh
            # matmuls/step read these at static addresses (a dynamic rhs
            # costs a FusedRegOps per matmul on the PE queue); the
            # t-indexed h history for emission is written by the Pool
            # engine off the critical path.
            h_pp = [[state.tile([128, 16], dt.bfloat16, name=f"hpp{d}{k}")
                     for k in range(2)] for d in range(2)]

            with (
                tc.tile_pool(name="jp", bufs=2, space="PSUM") as jp,
                tc.tile_pool(name="gp", bufs=4, space="PSUM") as gp,
                tc.tile_pool(name="ep", bufs=2, space="PSUM") as ep,
                tc.tile_pool(name="sp", bufs=4) as sp,
                tc.tile_pool(name="mp", bufs=2) as mp,
            ):
                with tc.For_i(0, reps, 1) as _rep:
                    # fresh state per rep
                    nc.vector.memset(h_pp[0][0][:], 0.0)
                    nc.vector.memset(h_pp[1][0][:], 0.0)
                    nc.vector.memset(c_f[:], 0.0)
                    nc.vector.memset(c_b[:], 0.0)

                    # ---- phase 1: P precompute ----
                    PW1 = 2 if NWIN % 2 == 0 else 1
                    with tc.For_i(0, 0 if skip_rest else NWIN, PW1) as w0:
                        for dw in range(PW1):
                            w = w0 + dw
                            for d in range(2):
                                P_d = P_f if d == 0 else P_b
                                for c in range(8):
                                    pp = jp.tile([128, NW], dt.float32,
                                                 tag="jp")
                                    nc.tensor.matmul(
                                        out=pp[:],
                                        lhsT=wTd(d, 0)[:,
                                                       c * 128:(c + 1) * 128],
                                        rhs=xT[:, ds(w * NW, NW)],
                                        start=True, stop=True)
                                    # alternate DVE/ACT so neither engine
                                    # serializes phase 1 (Pool cannot read
                                    # PSUM; ACT Identity shares the sigmoid
                                    # table set, so no table reloads)
                                    bcol = bias_sb[:, d * 8 + c:d * 8 + c + 1]
                                    if c % 2 == 0:
                                        nc.vector.tensor_scalar(
                                            out=P_d[:, ds(c * NT + w * NW,
                                                          NW)],
                                            in0=pp[:], scalar1=bcol,
                                            scalar2=None, op0=OP.add)
                                    else:
                                        nc.scalar.activation(
                                            out=P_d[:, ds(c * NT + w * NW,
                                                          NW)],
                                            in_=pp[:], func=AF.Identity,
                                            bias=bcol)

                    # ---- phase 2: scan ----
                    def scan_step(d, s, u):
                        """One LSTM step for direction d at scan index s
                        (a ScalarValue expression). fwd t=s; bwd t=L-1-s."""
                        P_d = P_f if d == 0 else P_b
                        h_d = h_f if d == 0 else h_b
                        c_d = c_f if d == 0 else c_b
                        if d == 0:
                            p_off = s * 8               # t = s
                            hw_off = s * 16 + 16        # slot t+1
                        else:
                            p_off = (L_eff - 1) * 8 - s * 8       # t = L-1-s
                            hw_off = (L_eff - 1) * 16 - s * 16    # slot t
                        hp = h_pp[d][u % 2]
                        hw = h_pp[d][(u + 1) % 2]
                        g_ps = gp.tile([128, 64], dt.float32, tag="g")
                        nc.tensor.matmul(
                            out=g_ps[:], lhsT=ident_s[:],
                            rhs=P_d.rearrange("p (c n) -> p c n", c=8)
                                 [:, :, ds(p_off, 8)],
                            start=True, stop=True)
                        for c in range(8):
                            for kt in (1, 2):
                                nc.tensor.matmul(
                                    out=g_ps[:, c * 8:(c + 1) * 8],
                                    lhsT=wTd(d, kt)[:, c * 128:(c + 1) * 128],
                                    rhs=hp[:, (kt - 1) * 8:kt * 8],
                                    start=False, stop=False,
                                    skip_group_check=True)
                        sg = sp.tile([128, 64], dt.float32, tag="s")
                        nc.scalar.activation(out=sg[:, 0:48],
                                             in_=g_ps[:, 0:48],
                                             func=AF.Sigmoid)
                        nc.scalar.activation(out=sg[:, 48:64],
                                             in_=g_ps[:, 48:64],
                                             func=AF.Tanh)
                        # f*c_prev -> scratch (must read c_d before overwrite)
                        nc.vector.tensor_tensor(out=sg[:, 16:32],
                                                in0=sg[:, 16:32],
                                                in1=c_d[:], op=OP.mult)
                        nc.vector.tensor_tensor(out=c_d[:], in0=sg[:, 0:16],
                                                in1=sg[:, 48:64], op=OP.mult)
                        nc.vector.tensor_tensor(out=c_d[:], in0=c_d[:],
                                                in1=sg[:, 16:32], op=OP.add)
                        nc.scalar.activation(out=sg[:, 48:64], in_=c_d[:],
                                             func=AF.Tanh)
                        nc.vector.tensor_tensor(out=hw[:],
                                                in0=sg[:, 32:48],
                                                in1=sg[:, 48:64], op=OP.mult)
                        # t-indexed history for emission: duplicate the
                        # multiply on the (idle) Pool engine, off the
                        # recurrence critical path.
                        nc.gpsimd.tensor_tensor(out=h_d[:, ds(hw_off, 16)],
                                                in0=sg[:, 32:48],
                                                in1=sg[:, 48:64], op=OP.mult)

                    with tc.For_i(0, 0 if skip_scan else L_eff, U,
                                  staggered_reset=staggered,
                                  hint_engines=hints) as s0:
                        for u in range(U):
                            scan_step(0, s0 + u, u)
                            scan_step(1, s0 + u, u)

                    # ---- phase 3: emission ----
                    for blk in range(0 if skip_rest else NEB):
                        t0 = blk * (EB // 8)
                        nt = EB // 8
                        eps = ep.tile([32, EB], dt.float32, tag="e")
                        for kt in range(4):
                            h_d = h_f if kt < 2 else h_b
                            c = kt % 2
                            off = 1 if kt < 2 else 0  # fwd slot t+1, bwd t
                            rhs = h_d.rearrange("p (t x) -> p t x", x=16)[
                                :, t0 + off:t0 + off + nt, c * 8:(c + 1) * 8]
                            nc.tensor.matmul(
                                out=eps[:],
                                lhsT=wlin_sb[:, kt * 32:(kt + 1) * 32],
                                rhs=rhs, start=(kt == 0), stop=(kt == 3))
                        esb = mp.tile([32, EB], dt.float32, tag="m")
                        nc.vector.tensor_copy(out=esb[:], in_=eps[:])
                        nc.sync.dma_start(
                            out=d_out[:, blk * EB:(blk + 1) * EB],
                            in_=esb[:])

                if timing:
                    tl = mp.tile([1, 16], dt.float32, tag="tl")
                    nc.sync.dma_start(out=tl[:], in_=d_out[0:1, 0:16])
                    nc.sync.dma_start(out=d_out_ext[:], in_=tl[:])

    nc.finalize()
    return nc


_CACHE = {}


def _get_nc(L_eff=L, reps=1, timing=False):
    key = (L_eff, reps, timing)
    if key not in _CACHE:
        _CACHE[key] = build_nc(L_eff, reps, timing)
    return _CACHE[key]


_RUNNERS = {}


def _make_runner(nc, n_cores):
    """Persistent jitted executor: the same bass2jax/PJRT path that
    run_bass_kernel_spmd takes under axon, but the jit/shard_map callable is
    built once and cached, so repeat kernel() calls skip the per-call
    retrace + XLA recompile + NEFF device reload (~2s each)."""
    import jax
    import numpy as _np
    from jax.sharding import Mesh, PartitionSpec
    import warnings
    try:
        with warnings.catch_warnings():
            warnings.simplefilter("ignore")
            from jax.experimental.shard_map import shard_map

            def _smap(f, mesh, in_specs, out_specs):
                return shard_map(f, mesh=mesh, in_specs=in_specs,
                                 out_specs=out_specs, check_rep=False)
    except ImportError:
        from jax import shard_map as _sm

        def _smap(f, mesh, in_specs, out_specs):
            return _sm(f, mesh=mesh, in_specs=in_specs,
                       out_specs=out_specs, check_vma=False)
    import concourse.mybir as mybir
    from concourse import bass2jax
    from concourse.bass2jax import _bass_exec_p, install_neuronx_cc_hook

    install_neuronx_cc_hook()
    partition_name = (nc.partition_id_tensor.name
                      if nc.partition_id_tensor else None)
    in_names, out_names, out_avals = [], [], []
    for alloc in nc.m.functions[0].allocations:
        if not isinstance(alloc, mybir.MemoryLocationSet):
            continue
        name = alloc.memorylocations[0].name
        if alloc.kind == "ExternalInput":
            if name != partition_name:
                in_names.append(name)
        elif alloc.kind == "ExternalOutput":
            out_names.append(name)
            out_avals.append(jax.core.ShapedArray(
                tuple(alloc.tensor_shape), mybir.dt.np(alloc.dtype)))
    n_params = len(in_names)
    all_in_names = list(in_names) + list(out_names)
    if partition_name is not None:
        all_in_names.append(partition_name)

    def _body(*args):
        operands = list(args)
        if partition_name is not None:
            operands.append(bass2jax.partition_id_tensor())
        outs = _bass_exec_p.bind(
            *operands,
            out_avals=tuple(out_avals),
            in_names=tuple(all_in_names),
            out_names=tuple(out_names),
            lowering_input_output_aliases=(),
            sim_require_finite=True,
            sim_require_nnan=True,
            nc=nc,
        )
        return tuple(outs)

    devices = jax.devices()[:n_cores]
    mesh = Mesh(_np.asarray(devices), ("core",))
    n_outs = len(out_avals)
    sharded = jax.jit(
        _smap(_body, mesh,
              (PartitionSpec("core"),) * (n_params + n_outs),
              (PartitionSpec("core"),) * n_outs),
        keep_unused=True,
    )

    # Device-resident input cache keyed by content hash: repeat kernel()
    # calls with unchanged weights/tokens skip the ~20MB tunnel re-upload.
    # (Transfer caching only — the device still executes every call.)
    import hashlib
    from jax.sharding import NamedSharding
    sharding = NamedSharding(mesh, PartitionSpec("core"))
    dev_cache = {}

    def _put(name, per_core):
        h = hashlib.blake2b(digest_size=16)
        for a in per_core:
            h.update(_np.ascontiguousarray(a).view(_np.uint8))
        key = (h.hexdigest(), tuple(per_core[0].shape))
        ent = dev_cache.get(name)
        if ent is not None and ent[0] == key:
            return ent[1]
        dev = jax.device_put(_np.concatenate(per_core, axis=0), sharding)
        dev_cache[name] = (key, dev)
        return dev

    def run(in_maps):
        concat_in = [
            _put(name, [_np.asarray(m[name]) for m in in_maps])
            for name in in_names
        ]
        if "zeros" not in dev_cache:
            dev_cache["zeros"] = [
                jax.device_put(
                    _np.zeros((n_cores * a.shape[0], *a.shape[1:]), a.dtype),
                    sharding)
                for a in out_avals
            ]
        out = sharded(*concat_in, *dev_cache["zeros"])
        return [
            {name: _np.asarray(out[i]).reshape(n_cores, *out_avals[i].shape)[c]
             for i, name in enumerate(out_names)}
            for c in range(n_cores)
        ]

    return run


def _run_spmd(nc, in_maps):
    key = id(nc)
    if key not in _RUNNERS:
        _RUNNERS[key] = _make_runner(nc, NCORES)
    return _RUNNERS[key](in_maps)


def _assemble(outs, inputs, L_eff=L):
    """Host CRF broadcast: out[b,l,i,j] = emis[b,l,j] + trans[i,j] + b_lin[j]."""
    import concurrent.futures as cf

    M = (np.asarray(inputs["transition"], np.float32)
         + np.asarray(inputs["b_lin"], np.float32)[None, :])  # [32, 32]
    res = np.empty((B, L_eff, OUT, OUT), np.float32)

    def fill(c):
        emis = outs[c].reshape(32, L_eff, BC).transpose(2, 1, 0)  # [8, L, 32]
        np.add(emis[:, :, None, :], M[None, None, :, :],
               out=res[c * BC:(c + 1) * BC])

    with cf.ThreadPoolExecutor(max_workers=8) as ex:
        list(ex.map(fill, range(NCORES)))
    return res


def kernel(**inputs):
    nc = _get_nc(L, 1, False)
    in_maps = _host_prep(inputs, L)
    try:
        results = _run_spmd(nc, in_maps)
    except Exception:
        # fallback: per-call path through bass_utils (retraces every call)
        from concourse.bass_utils import run_bass_kernel_spmd
        results = run_bass_kernel_spmd(nc, in_maps,
                                       list(range(NCORES))).results
    outs = [results[c]["out"] for c in range(NCORES)]
    return _assemble(outs, inputs, L)


if __name__ == "__main__":
    nc = build_nc(64)
    print("built OK")


# revision 50
# speedup vs baseline: 1443.5481x; 1.0518x over previous
"""BiLSTM-CRF Trainium2 kernel (v2: hardware-looped scan, minimal I/O).

Full-input contract: kernel(**inputs) takes the unsharded numpy inputs and
returns the full [64, 512, 32, 32] float32 output. Internally shards the
batch (64) across 8 NeuronCores (8 sentences per core), runs a Bass kernel
SPMD, and assembles the output on host.

Device work per core (all phases inside a For_i rep loop so benchmark
variants repeat the body without growing the NEFF):
  phase 1  For_i over 16 windows: P = Wih @ x (+gate bias) for all 512
           steps, both directions, written to SBUF-resident P_f/P_b
           (bf16, c-major layout [128, c*4096 + t*8 + b]).
  phase 2  For_i over 512 steps (unroll 4): both LSTM directions per
           iteration. Gates [128, 64] PSUM accumulate: identity matmul
           initializes with P_t, 16 small Whh matmuls accumulate the
           recurrent term. Chunk order [i0 i1 f0 f1 o0 o1 g0 g1] so
           sigmoid(i,f,o) is one ACT op. h stored bf16 at slot t+1 (fwd)
           / t (bwd) with zero boundary slots -> no step-0 special case.
  phase 3  emission matmul emisT[32, t*8+b] = W_lin^T-tiles @ h tiles,
           DMA'd straight from PSUM to DRAM [32, 4096] f32.

Host does the embedding gather (-> bf16 xT upload, ~1MB/core instead of a
15MB replicated table) and the CRF broadcast add
out[b,l,i,j] = emis[b,l,j] + transition[i,j] + b_lin[j] (134MB never
crosses the device tunnel; only 0.5MB of emissions per core does).
"""

import numpy as np

VOCAB, EMB, HID, OUT = 30000, 128, 256, 32
B, L = 64, 512
NCORES = 8
BC = B // NCORES  # batch per core = 8


def _host_prep(inputs, L_eff=L):
    """Prepare per-core in_maps (list of dicts) from full inputs."""
    import ml_dtypes

    sents = np.asarray(inputs["sents_tensor"]).astype(np.int32)  # [B, L]
    emb = np.asarray(inputs["embedding"]).astype(np.float32)  # [V, E]

    # gate permutation: torch order i,f,g,o -> ours i,f,o,g
    perm = np.concatenate([np.arange(0, 512), np.arange(768, 1024),
                           np.arange(512, 768)])

    def mk_wT(Wih, Whh, bih, bhh):
        Wih = np.asarray(Wih, np.float32)[perm]  # [1024, 128]
        Whh = np.asarray(Whh, np.float32)[perm]  # [1024, 256]
        wT = np.stack([
            np.ascontiguousarray(Wih.T),              # [128, 1024]
            np.ascontiguousarray(Whh[:, :128].T),     # [128, 1024]
            np.ascontiguousarray(Whh[:, 128:].T),     # [128, 1024]
        ])  # [3, 128, 1024]
        b = (np.asarray(bih, np.float32) + np.asarray(bhh, np.float32))[perm]
        b_sb = np.ascontiguousarray(b.reshape(8, 128).T)  # [128, 8] col=chunk
        return wT, b_sb

    wT_f, b_f = mk_wT(inputs["Wih_f"], inputs["Whh_f"], inputs["bih_f"],
                      inputs["bhh_f"])
    wT_b, b_b = mk_wT(inputs["Wih_b"], inputs["Whh_b"], inputs["bih_b"],
                      inputs["bhh_b"])
    wT = np.stack([wT_f, wT_b]).astype(ml_dtypes.bfloat16)  # [2,3,128,1024]
    bias = np.ascontiguousarray(
        np.concatenate([b_f, b_b], axis=1))  # [128, 16]: cols 0:8 f, 8:16 b

    W_lin = np.asarray(inputs["W_lin"], np.float32)      # [32, 2H]
    WlinT = np.ascontiguousarray(W_lin.T)                # [512, 32]
    wl_pm = np.ascontiguousarray(
        WlinT.reshape(4, 128, 32).transpose(1, 0, 2).reshape(128, 128)
    ).astype(ml_dtypes.bfloat16)                         # [128, 4*32]

    wT_pm = np.ascontiguousarray(
        wT.transpose(2, 0, 1, 3).reshape(128, -1))       # [128, 6*1024]
    c2 = np.concatenate([wT_pm, wl_pm], axis=1)          # [128, 6272] bf16

    # x gather: [B, L, E] fp32 -> per-core xT [128, L*8] bf16, col = t*8+b
    x = emb[sents[:, :L_eff]]  # [B, L_eff, 128] fp32
    in_maps = []
    for c in range(NCORES):
        xc = x[c * BC:(c + 1) * BC]                      # [8, L_eff, 128]
        xT = np.ascontiguousarray(
            xc.transpose(2, 1, 0).reshape(128, L_eff * BC)
        ).astype(ml_dtypes.bfloat16)                     # [128, L*8]
        in_maps.append({
            "c2": np.ascontiguousarray(c2),
            "c4": bias,
            "xt": xT,
        })
    return in_maps


def build_nc(L_eff=L, reps=1, timing=False, U=8, staggered=False,
             hints=(), skip_scan=False, skip_rest=False):
    """Build the Bass program (identical for every core).

    reps>1 repeats the compute body (P-precompute + scan + emission) via
    the outer For_i; the NEFF size does not grow with reps. timing=True
    swaps the big external tensors (xt in, emis out) for internal DRAM so
    benchmark calls transfer almost nothing over the axon tunnel.
    """
    import concourse.bass as bass
    import concourse.mybir as mybir
    import concourse.tile as tile
    from concourse.bacc import Bacc
    from concourse.bass import ds
    from concourse.masks import make_identity

    dt = mybir.dt
    AF = mybir.ActivationFunctionType
    OP = mybir.AluOpType

    NT = L_eff * BC            # tokens per direction (cols of xT)
    WIN = 32 if L_eff >= 32 else L_eff
    NWIN = L_eff // WIN
    NW = WIN * BC              # cols per P window chunk
    if L_eff % U != 0:
        U = 1                  # scan unroll
    EB = 512 if L_eff >= 64 else NT  # emission block cols
    NEB = NT // EB

    nc = Bacc()

    d_c2 = nc.declare_dram_parameter("c2", [128, 6 * 1024 + 4 * 32],
                                     dt.bfloat16, False)
    d_c4 = nc.declare_dram_parameter("c4", [128, 16], dt.float32, False)
    if timing:
        d_xt = None
        d_out = nc.dram_tensor("outt", [32, NT], dt.float32)
        d_out_ext = nc.declare_dram_parameter("out", [1, 16], dt.float32,
                                              isOutput=True)
    else:
        d_xt = nc.declare_dram_parameter("xt", [128, NT], dt.bfloat16, False)
        d_out = nc.declare_dram_parameter("out", [32, NT], dt.float32,
                                          isOutput=True)
        d_out_ext = None

    with tile.TileContext(nc) as tc:
        with (
            tc.tile_pool(name="const", bufs=1) as const,
            tc.tile_pool(name="state", bufs=1) as state,
        ):
            ident = const.tile([128, 128], dt.float32)
            make_identity(nc, ident[:])
            ident_s = const.tile([128, 128], dt.bfloat16)
            nc.vector.tensor_copy(out=ident_s[:], in_=ident[:])
            c2_sb = const.tile([128, 6 * 1024 + 4 * 32], dt.bfloat16)
            nc.sync.dma_start(out=c2_sb[:], in_=d_c2[:])
            wT_sb = c2_sb[:, 0:6 * 1024]
            wlin_sb = c2_sb[:, 6 * 1024:]
            c4_sb = const.tile([128, 16], dt.float32)
            nc.sync.dma_start(out=c4_sb[:], in_=d_c4[:])
            bias_sb = c4_sb
            # dummy activation so the sigmoid/tanh table set is resident on
            # every path into the scan loop -> the table-load pass can keep
            # InstLoadActFuncSet out of the loop body
            warm = const.tile([128, 1], dt.float32)
            nc.scalar.activation(out=warm[:], in_=ident[:, 0:1],
                                 func=AF.Sigmoid)

            def wTd(d, kt):  # [128, 1024] weight K-tile
                off = (d * 3 + kt) * 1024
                return wT_sb[:, off:off + 1024]

            xT = state.tile([128, NT], dt.bfloat16)
            if timing:
                nc.vector.memset(xT[:], 0.0)
            else:
                nc.sync.dma_start(out=xT[:], in_=d_xt[:])

            # P layout per dir: [128, c*NT + t*8 + b], bf16
            P_f = state.tile([128, 8 * NT], dt.bfloat16)
            P_b = state.tile([128, 8 * NT], dt.bfloat16)
            # h slots: fwd slot t+1 = h_t (slot 0 = zeros);
            #          bwd slot t   = h_t (slot L = zeros)
            h_f = state.tile([128, (L_eff + 1) * 16], dt.bfloat16)
            h_b = state.tile([128, (L_eff + 1) * 16], dt.bfloat16)
            c_f = state.tile([128, 16], dt.float32)
            c_b = state.tile([128, 16], dt.float32)
            # static ping-pong h tiles for the recurrence: the 16 Whh
            # matmuls/step read these at static addresses (a dynamic rhs
            # costs a FusedRegOps per matmul on the PE queue); the
            # t-indexed h history for emission is written by the Pool
            # engine off the critical path.
            h_pp = [[state.tile([128, 16], dt.bfloat16, name=f"hpp{d}{k}")
                     for k in range(2)] for d in range(2)]

            with (
                tc.tile_pool(name="jp", bufs=2, space="PSUM") as jp,
                tc.tile_pool(name="gp", bufs=4, space="PSUM") as gp,
                tc.tile_pool(name="ep", bufs=2, space="PSUM") as ep,
                tc.tile_pool(name="sp", bufs=4) as sp,
                tc.tile_pool(name="mp", bufs=2) as mp,
            ):
                with tc.For_i(0, reps, 1) as _rep:
                    # fresh state per rep
                    nc.vector.memset(h_pp[0][0][:], 0.0)
                    nc.vector.memset(h_pp[1][0][:], 0.0)
                    nc.vector.memset(c_f[:], 0.0)
                    nc.vector.memset(c_b[:], 0.0)

                    # ---- phase 1: P precompute ----
                    PW1 = 2 if NWIN % 2 == 0 else 1
                    with tc.For_i(0, 0 if skip_rest else NWIN, PW1) as w0:
                        for dw in range(PW1):
                            w = w0 + dw
                            for d in range(2):
                                P_d = P_f if d == 0 else P_b
                                for c in range(8):
                                    pp = jp.tile([128, NW], dt.float32,
                                                 tag="jp")
                                    nc.tensor.matmul(
                                        out=pp[:],
                                        lhsT=wTd(d, 0)[:,
                                                       c * 128:(c + 1) * 128],
                                        rhs=xT[:, ds(w * NW, NW)],
                                        start=True, stop=True)
                                    # alternate DVE/ACT so neither engine
                                    # serializes phase 1 (Pool cannot read
                                    # PSUM; ACT Identity shares the sigmoid
                                    # table set, so no table reloads)
                                    bcol = bias_sb[:, d * 8 + c:d * 8 + c + 1]
                                    if c % 2 == 0:
                                        nc.vector.tensor_scalar(
                                            out=P_d[:, ds(c * NT + w * NW,
                                                          NW)],
                                            in0=pp[:], scalar1=bcol,
                                            scalar2=None, op0=OP.add)
                                    else:
                                        nc.scalar.activation(
                                            out=P_d[:, ds(c * NT + w * NW,
                                                          NW)],
                                            in_=pp[:], func=AF.Identity,
                                            bias=bcol)

                    # ---- phase 2: scan ----
                    # Each step is split into a gate sub-phase (matmuls,
                    # sigmoid/tanh, c update) and a c/h sub-phase (tanh(c),
                    # h writes), emitted for BOTH directions phase-by-phase.
                    # Engine queues are strictly in-order, so emitting
                    # d0's tanh(c) before d1's sigmoid would head-of-line
                    # block d1's (long-ready) sigmoid behind d0's whole
                    # DVE c-chain.
                    def scan_step_a(d, s, u):
                        """Gates + c update for direction d at scan index s
                        (a ScalarValue expression). fwd t=s; bwd t=L-1-s."""
                        P_d = P_f if d == 0 else P_b
                        c_d = c_f if d == 0 else c_b
                        p_off = (s * 8 if d == 0 else
                                 (L_eff - 1) * 8 - s * 8)
                        hp = h_pp[d][u % 2]
                        g_ps = gp.tile([128, 64], dt.float32, tag="g")
                        nc.tensor.matmul(
                            out=g_ps[:], lhsT=ident_s[:],
                            rhs=P_d.rearrange("p (c n) -> p c n", c=8)
                                 [:, :, ds(p_off, 8)],
                            start=True, stop=True)
                        for c in range(8):
                            for kt in (1, 2):
                                nc.tensor.matmul(
                                    out=g_ps[:, c * 8:(c + 1) * 8],
                                    lhsT=wTd(d, kt)[:, c * 128:(c + 1) * 128],
                                    rhs=hp[:, (kt - 1) * 8:kt * 8],
                                    start=False, stop=False,
                                    skip_group_check=True)
                        sg = sp.tile([128, 64], dt.float32, tag="s")
                        nc.scalar.activation(out=sg[:, 0:48],
                                             in_=g_ps[:, 0:48],
                                             func=AF.Sigmoid)
                        nc.scalar.activation(out=sg[:, 48:64],
                                             in_=g_ps[:, 48:64],
                                             func=AF.Tanh)
                        # f*c_prev -> scratch (must read c_d before overwrite)
                        nc.vector.tensor_tensor(out=sg[:, 16:32],
                                                in0=sg[:, 16:32],
                                                in1=c_d[:], op=OP.mult)
                        nc.vector.tensor_tensor(out=c_d[:], in0=sg[:, 0:16],
                                                in1=sg[:, 48:64], op=OP.mult)
                        nc.vector.tensor_tensor(out=c_d[:], in0=c_d[:],
                                                in1=sg[:, 16:32], op=OP.add)
                        return sg

                    def scan_step_b(d, s, u, sg):
                        """tanh(c) + h writes for direction d."""
                        h_d = h_f if d == 0 else h_b
                        c_d = c_f if d == 0 else c_b
                        hw_off = (s * 16 + 16 if d == 0 else
                                  (L_eff - 1) * 16 - s * 16)
                        hw = h_pp[d][(u + 1) % 2]
                        nc.scalar.activation(out=sg[:, 48:64], in_=c_d[:],
                                             func=AF.Tanh)
                        nc.vector.tensor_tensor(out=hw[:],
                                                in0=sg[:, 32:48],
                                                in1=sg[:, 48:64], op=OP.mult)
                        # t-indexed history for emission: duplicate the
                        # multiply on the (idle) Pool engine, off the
                        # recurrence critical path.
                        nc.gpsimd.tensor_tensor(out=h_d[:, ds(hw_off, 16)],
                                                in0=sg[:, 32:48],
                                                in1=sg[:, 48:64], op=OP.mult)

                    with tc.For_i(0, 0 if skip_scan else L_eff, U,
                                  staggered_reset=staggered,
                                  hint_engines=hints) as s0:
                        for u in range(U):
                            sg0 = scan_step_a(0, s0 + u, u)
                            sg1 = scan_step_a(1, s0 + u, u)
                            scan_step_b(0, s0 + u, u, sg0)
                            scan_step_b(1, s0 + u, u, sg1)

                    # ---- phase 3: emission ----
                    for blk in range(0 if skip_rest else NEB):
                        t0 = blk * (EB // 8)
                        nt = EB // 8
                        eps = ep.tile([32, EB], dt.float32, tag="e")
                        for kt in range(4):
                            h_d = h_f if kt < 2 else h_b
                            c = kt % 2
                            off = 1 if kt < 2 else 0  # fwd slot t+1, bwd t
                            rhs = h_d.rearrange("p (t x) -> p t x", x=16)[
                                :, t0 + off:t0 + off + nt, c * 8:(c + 1) * 8]
                            nc.tensor.matmul(
                                out=eps[:],
                                lhsT=wlin_sb[:, kt * 32:(kt + 1) * 32],
                                rhs=rhs, start=(kt == 0), stop=(kt == 3))
                        esb = mp.tile([32, EB], dt.float32, tag="m")
                        nc.vector.tensor_copy(out=esb[:], in_=eps[:])
                        nc.sync.dma_start(
                            out=d_out[:, blk * EB:(blk + 1) * EB],
                            in_=esb[:])

                if timing:
                    tl = mp.tile([1, 16], dt.float32, tag="tl")
                    nc.sync.dma_start(out=tl[:], in_=d_out[0:1, 0:16])
                    nc.sync.dma_start(out=d_out_ext[:], in_=tl[:])

    nc.finalize()
    return nc


_CACHE = {}


def _get_nc(L_eff=L, reps=1, timing=False):
    key = (L_eff, reps, timing)
    if key not in _CACHE:
        _CACHE[key] = build_nc(L_eff, reps, timing)
    return _CACHE[key]


_RUNNERS = {}


def _make_runner(nc, n_cores):
    """Persistent jitted executor: the same bass2jax/PJRT path that
    run_bass_kernel_spmd takes under axon, but the jit/shard_map callable is
    built once and cached, so repeat kernel() calls skip the per-call
    retrace + XLA recompile + NEFF device reload (~2s each)."""
    import jax
    import numpy as _np
    from jax.sharding import Mesh, PartitionSpec
    import warnings
    try:
        with warnings.catch_warnings():
            warnings.simplefilter("ignore")
            from jax.experimental.shard_map import shard_map

            def _smap(f, mesh, in_specs, out_specs):
                return shard_map(f, mesh=mesh, in_specs=in_specs,
                                 out_specs=out_specs, check_rep=False)
    except ImportError:
        from jax import shard_map as _sm

        def _smap(f, mesh, in_specs, out_specs):
            return _sm(f, mesh=mesh, in_specs=in_specs,
                       out_specs=out_specs, check_vma=False)
    import concourse.mybir as mybir
    from concourse import bass2jax
    from concourse.bass2jax import _bass_exec_p, install_neuronx_cc_hook

    install_neuronx_cc_hook()
    partition_name = (nc.partition_id_tensor.name
                      if nc.partition_id_tensor else None)
    in_names, out_names, out_avals = [], [], []
    for alloc in nc.m.functions[0].allocations:
        if not isinstance(alloc, mybir.MemoryLocationSet):
            continue
        name = alloc.memorylocations[0].name
        if alloc.kind == "ExternalInput":
            if name != partition_name:
                in_names.append(name)
        elif alloc.kind == "ExternalOutput":
            out_names.append(name)
            out_avals.append(jax.core.ShapedArray(
                tuple(alloc.tensor_shape), mybir.dt.np(alloc.dtype)))
    n_params = len(in_names)
    all_in_names = list(in_names) + list(out_names)
    if partition_name is not None:
        all_in_names.append(partition_name)

    def _body(*args):
        operands = list(args)
        if partition_name is not None:
            operands.append(bass2jax.partition_id_tensor())
        outs = _bass_exec_p.bind(
            *operands,
            out_avals=tuple(out_avals),
            in_names=tuple(all_in_names),
            out_names=tuple(out_names),
            lowering_input_output_aliases=(),
            sim_require_finite=True,
            sim_require_nnan=True,
            nc=nc,
        )
        return tuple(outs)

    devices = jax.devices()[:n_cores]
    mesh = Mesh(_np.asarray(devices), ("core",))
    n_outs = len(out_avals)
    sharded = jax.jit(
        _smap(_body, mesh,
              (PartitionSpec("core"),) * (n_params + n_outs),
              (PartitionSpec("core"),) * n_outs),
        keep_unused=True,
    )

    # Device-resident input cache keyed by content hash: repeat kernel()
    # calls with unchanged weights/tokens skip the ~20MB tunnel re-upload.
    # (Transfer caching only — the device still executes every call.)
    import hashlib
    from jax.sharding import NamedSharding
    sharding = NamedSharding(mesh, PartitionSpec("core"))
    dev_cache = {}

    def _put(name, per_core):
        h = hashlib.blake2b(digest_size=16)
        for a in per_core:
            h.update(_np.ascontiguousarray(a).view(_np.uint8))
        key = (h.hexdigest(), tuple(per_core[0].shape))
        ent = dev_cache.get(name)
        if ent is not None and ent[0] == key:
            return ent[1]
        dev = jax.device_put(_np.concatenate(per_core, axis=0), sharding)
        dev_cache[name] = (key, dev)
        return dev

    def run(in_maps):
        concat_in = [
            _put(name, [_np.asarray(m[name]) for m in in_maps])
            for name in in_names
        ]
        if "zeros" not in dev_cache:
            dev_cache["zeros"] = [
                jax.device_put(
                    _np.zeros((n_cores * a.shape[0], *a.shape[1:]), a.dtype),
                    sharding)
                for a in out_avals
            ]
        out = sharded(*concat_in, *dev_cache["zeros"])
        return [
            {name: _np.asarray(out[i]).reshape(n_cores, *out_avals[i].shape)[c]
             for i, name in enumerate(out_names)}
            for c in range(n_cores)
        ]

    return run


def _run_spmd(nc, in_maps):
    key = id(nc)
    if key not in _RUNNERS:
        _RUNNERS[key] = _make_runner(nc, NCORES)
    return _RUNNERS[key](in_maps)


def _assemble(outs, inputs, L_eff=L):
    """Host CRF broadcast: out[b,l,i,j] = emis[b,l,j] + trans[i,j] + b_lin[j]."""
    import concurrent.futures as cf

    M = (np.asarray(inputs["transition"], np.float32)
         + np.asarray(inputs["b_lin"], np.float32)[None, :])  # [32, 32]
    res = np.empty((B, L_eff, OUT, OUT), np.float32)

    def fill(c):
        emis = outs[c].reshape(32, L_eff, BC).transpose(2, 1, 0)  # [8, L, 32]
        np.add(emis[:, :, None, :], M[None, None, :, :],
               out=res[c * BC:(c + 1) * BC])

    with cf.ThreadPoolExecutor(max_workers=8) as ex:
        list(ex.map(fill, range(NCORES)))
    return res


def kernel(**inputs):
    nc = _get_nc(L, 1, False)
    in_maps = _host_prep(inputs, L)
    try:
        results = _run_spmd(nc, in_maps)
    except Exception:
        # fallback: per-call path through bass_utils (retraces every call)
        from concourse.bass_utils import run_bass_kernel_spmd
        results = run_bass_kernel_spmd(nc, in_maps,
                                       list(range(NCORES))).results
    outs = [results[c]["out"] for c in range(NCORES)]
    return _assemble(outs, inputs, L)


if __name__ == "__main__":
    nc = build_nc(64)
    print("built OK")


# revision 55
# speedup vs baseline: 1784.1778x; 1.2360x over previous
"""BiLSTM-CRF Trainium2 kernel (v2: hardware-looped scan, minimal I/O).

Full-input contract: kernel(**inputs) takes the unsharded numpy inputs and
returns the full [64, 512, 32, 32] float32 output. Internally shards the
batch (64) across 8 NeuronCores (8 sentences per core), runs a Bass kernel
SPMD, and assembles the output on host.

Device work per core (all phases inside a For_i rep loop so benchmark
variants repeat the body without growing the NEFF):
  phase 1  For_i over 16 windows: P = Wih @ x (+gate bias) for all 512
           steps, both directions, written to SBUF-resident P_f/P_b
           (bf16, c-major layout [128, c*4096 + t*8 + b]).
  phase 2  For_i over 512 steps (unroll 4): both LSTM directions per
           iteration. Gates [128, 64] PSUM accumulate: identity matmul
           initializes with P_t, 16 small Whh matmuls accumulate the
           recurrent term. Chunk order [i0 i1 f0 f1 o0 o1 g0 g1] so
           sigmoid(i,f,o) is one ACT op. h stored bf16 at slot t+1 (fwd)
           / t (bwd) with zero boundary slots -> no step-0 special case.
  phase 3  emission matmul emisT[32, t*8+b] = W_lin^T-tiles @ h tiles,
           DMA'd straight from PSUM to DRAM [32, 4096] f32.

Host does the embedding gather (-> bf16 xT upload, ~1MB/core instead of a
15MB replicated table) and the CRF broadcast add
out[b,l,i,j] = emis[b,l,j] + transition[i,j] + b_lin[j] (134MB never
crosses the device tunnel; only 0.5MB of emissions per core does).
"""

import numpy as np

VOCAB, EMB, HID, OUT = 30000, 128, 256, 32
B, L = 64, 512
NCORES = 8
BC = B // NCORES  # batch per core = 8


def _host_prep(inputs, L_eff=L):
    """Prepare per-core in_maps (list of dicts) from full inputs."""
    import ml_dtypes

    sents = np.asarray(inputs["sents_tensor"]).astype(np.int32)  # [B, L]
    emb = np.asarray(inputs["embedding"]).astype(np.float32)  # [V, E]

    # gate permutation: torch order i,f,g,o -> ours i,f,o,g
    perm = np.concatenate([np.arange(0, 512), np.arange(768, 1024),
                           np.arange(512, 768)])

    def mk_wT(Wih, Whh, bih, bhh):
        Wih = np.asarray(Wih, np.float32)[perm]  # [1024, 128]
        Whh = np.asarray(Whh, np.float32)[perm]  # [1024, 256]
        wT = np.stack([
            np.ascontiguousarray(Wih.T),              # [128, 1024]
            np.ascontiguousarray(Whh[:, :128].T),     # [128, 1024]
            np.ascontiguousarray(Whh[:, 128:].T),     # [128, 1024]
        ])  # [3, 128, 1024]
        b = (np.asarray(bih, np.float32) + np.asarray(bhh, np.float32))[perm]
        b_sb = np.ascontiguousarray(b.reshape(8, 128).T)  # [128, 8] col=chunk
        return wT, b_sb

    wT_f, b_f = mk_wT(inputs["Wih_f"], inputs["Whh_f"], inputs["bih_f"],
                      inputs["bhh_f"])
    wT_b, b_b = mk_wT(inputs["Wih_b"], inputs["Whh_b"], inputs["bih_b"],
                      inputs["bhh_b"])
    wT = np.stack([wT_f, wT_b]).astype(ml_dtypes.bfloat16)  # [2,3,128,1024]
    bias = np.ascontiguousarray(
        np.concatenate([b_f, b_b], axis=1))  # [128, 16]: cols 0:8 f, 8:16 b

    W_lin = np.asarray(inputs["W_lin"], np.float32)      # [32, 2H]
    WlinT = np.ascontiguousarray(W_lin.T)                # [512, 32]
    wl_pm = np.ascontiguousarray(
        WlinT.reshape(4, 128, 32).transpose(1, 0, 2).reshape(128, 128)
    ).astype(ml_dtypes.bfloat16)                         # [128, 4*32]

    wT_pm = np.ascontiguousarray(
        wT.transpose(2, 0, 1, 3).reshape(128, -1))       # [128, 6*1024]
    c2 = np.concatenate([wT_pm, wl_pm], axis=1)          # [128, 6272] bf16

    # x gather: [B, L, E] fp32 -> per-core xT [128, L*8] bf16, col = t*8+b
    x = emb[sents[:, :L_eff]]  # [B, L_eff, 128] fp32
    in_maps = []
    for c in range(NCORES):
        xc = x[c * BC:(c + 1) * BC]                      # [8, L_eff, 128]
        xT = np.ascontiguousarray(
            xc.transpose(2, 1, 0).reshape(128, L_eff * BC)
        ).astype(ml_dtypes.bfloat16)                     # [128, L*8]
        in_maps.append({
            "c2": np.ascontiguousarray(c2),
            "c4": bias,
            "xt": xT,
        })
    return in_maps


def build_nc(L_eff=L, reps=1, timing=False, U=8, staggered=False,
             hints=(), skip_scan=False, skip_rest=False):
    """Build the Bass program (identical for every core).

    reps>1 repeats the compute body (P-precompute + scan + emission) via
    the outer For_i; the NEFF size does not grow with reps. timing=True
    swaps the big external tensors (xt in, emis out) for internal DRAM so
    benchmark calls transfer almost nothing over the axon tunnel.
    """
    import concourse.bass as bass
    import concourse.mybir as mybir
    import concourse.tile as tile
    from concourse.bacc import Bacc
    from concourse.bass import ds
    from concourse.masks import make_identity

    dt = mybir.dt
    AF = mybir.ActivationFunctionType
    OP = mybir.AluOpType

    NT = L_eff * BC            # tokens per direction (cols of xT)
    WIN = 32 if L_eff >= 32 else L_eff
    NWIN = L_eff // WIN
    NW = WIN * BC              # cols per P window chunk
    if L_eff % U != 0:
        U = 1                  # scan unroll
    EB = 512 if L_eff >= 64 else NT  # emission block cols
    NEB = NT // EB

    nc = Bacc()

    d_c2 = nc.declare_dram_parameter("c2", [128, 6 * 1024 + 4 * 32],
                                     dt.bfloat16, False)
    d_c4 = nc.declare_dram_parameter("c4", [128, 16], dt.float32, False)
    if timing:
        d_xt = None
        d_out = nc.dram_tensor("outt", [32, NT], dt.float32)
        d_out_ext = nc.declare_dram_parameter("out", [1, 16], dt.float32,
                                              isOutput=True)
    else:
        d_xt = nc.declare_dram_parameter("xt", [128, NT], dt.bfloat16, False)
        d_out = nc.declare_dram_parameter("out", [32, NT], dt.float32,
                                          isOutput=True)
        d_out_ext = None

    with tile.TileContext(nc) as tc:
        with (
            tc.tile_pool(name="const", bufs=1) as const,
            tc.tile_pool(name="state", bufs=1) as state,
        ):
            ident = const.tile([128, 128], dt.float32)
            make_identity(nc, ident[:])
            ident_s = const.tile([128, 128], dt.bfloat16)
            nc.vector.tensor_copy(out=ident_s[:], in_=ident[:])
            c2_sb = const.tile([128, 6 * 1024 + 4 * 32], dt.bfloat16)
            nc.sync.dma_start(out=c2_sb[:], in_=d_c2[:])
            wT_sb = c2_sb[:, 0:6 * 1024]
            wlin_sb = c2_sb[:, 6 * 1024:]
            c4_sb = const.tile([128, 16], dt.float32)
            nc.sync.dma_start(out=c4_sb[:], in_=d_c4[:])
            bias_sb = c4_sb
            # dummy activation so the sigmoid/tanh table set is resident on
            # every path into the scan loop -> the table-load pass can keep
            # InstLoadActFuncSet out of the loop body
            warm = const.tile([128, 1], dt.float32)
            nc.scalar.activation(out=warm[:], in_=ident[:, 0:1],
                                 func=AF.Sigmoid)

            def wTd(d, kt):  # [128, 1024] weight K-tile
                off = (d * 3 + kt) * 1024
                return wT_sb[:, off:off + 1024]

            xT = state.tile([128, NT], dt.bfloat16)
            if timing:
                nc.vector.memset(xT[:], 0.0)
            else:
                nc.sync.dma_start(out=xT[:], in_=d_xt[:])

            # P layout per dir: [128, c*NT + t*8 + b], bf16
            P_f = state.tile([128, 8 * NT], dt.bfloat16)
            P_b = state.tile([128, 8 * NT], dt.bfloat16)
            # h slots: fwd slot t+1 = h_t (slot 0 = zeros);
            #          bwd slot t   = h_t (slot L = zeros)
            h_f = state.tile([128, (L_eff + 1) * 16], dt.bfloat16)
            h_b = state.tile([128, (L_eff + 1) * 16], dt.bfloat16)
            c_f = state.tile([128, 16], dt.float32)
            c_b = state.tile([128, 16], dt.float32)
            # static ping-pong h tiles for the recurrence: the 16 Whh
            # matmuls/step read these at static addresses (a dynamic rhs
            # costs a FusedRegOps per matmul on the PE queue); the
            # t-indexed h history for emission is written by the Pool
            # engine off the critical path.
            h_pp = [[state.tile([128, 16], dt.bfloat16, name=f"hpp{d}{k}")
                     for k in range(2)] for d in range(2)]

            with (
                tc.tile_pool(name="jp", bufs=2, space="PSUM") as jp,
                tc.tile_pool(name="gp", bufs=4, space="PSUM") as gp,
                tc.tile_pool(name="ep", bufs=2, space="PSUM") as ep,
                tc.tile_pool(name="sp", bufs=4) as sp,
                tc.tile_pool(name="mp", bufs=2) as mp,
            ):
                with tc.For_i(0, reps, 1) as _rep:
                    # fresh state per rep
                    nc.vector.memset(h_pp[0][0][:], 0.0)
                    nc.vector.memset(h_pp[1][0][:], 0.0)
                    nc.vector.memset(c_f[:], 0.0)
                    nc.vector.memset(c_b[:], 0.0)

                    # ---- phase 1: P precompute ----
                    PW1 = 2 if NWIN % 2 == 0 else 1
                    with tc.For_i(0, 0 if skip_rest else NWIN, PW1) as w0:
                        for dw in range(PW1):
                            w = w0 + dw
                            for d in range(2):
                                P_d = P_f if d == 0 else P_b
                                for c in range(8):
                                    pp = jp.tile([128, NW], dt.float32,
                                                 tag="jp")
                                    nc.tensor.matmul(
                                        out=pp[:],
                                        lhsT=wTd(d, 0)[:,
                                                       c * 128:(c + 1) * 128],
                                        rhs=xT[:, ds(w * NW, NW)],
                                        start=True, stop=True)
                                    # alternate DVE/ACT so neither engine
                                    # serializes phase 1 (Pool cannot read
                                    # PSUM; ACT Identity shares the sigmoid
                                    # table set, so no table reloads)
                                    bcol = bias_sb[:, d * 8 + c:d * 8 + c + 1]
                                    if c % 2 == 0:
                                        nc.vector.tensor_scalar(
                                            out=P_d[:, ds(c * NT + w * NW,
                                                          NW)],
                                            in0=pp[:], scalar1=bcol,
                                            scalar2=None, op0=OP.add)
                                    else:
                                        nc.scalar.activation(
                                            out=P_d[:, ds(c * NT + w * NW,
                                                          NW)],
                                            in_=pp[:], func=AF.Identity,
                                            bias=bcol)

                    # ---- phase 2: scan ----
                    # Each step is split into a gate sub-phase (matmuls,
                    # sigmoid/tanh, c update) and a c/h sub-phase (tanh(c),
                    # h writes), emitted for BOTH directions phase-by-phase.
                    # Engine queues are strictly in-order, so emitting
                    # d0's tanh(c) before d1's sigmoid would head-of-line
                    # block d1's (long-ready) sigmoid behind d0's whole
                    # DVE c-chain.
                    def scan_step_a(d, s, u):
                        """Gates + c update for direction d at scan index s
                        (a ScalarValue expression). fwd t=s; bwd t=L-1-s."""
                        P_d = P_f if d == 0 else P_b
                        c_d = c_f if d == 0 else c_b
                        p_off = (s * 8 if d == 0 else
                                 (L_eff - 1) * 8 - s * 8)
                        hp = h_pp[d][u % 2]
                        g_ps = gp.tile([128, 64], dt.float32, tag="g")
                        nc.tensor.matmul(
                            out=g_ps[:], lhsT=ident_s[:],
                            rhs=P_d.rearrange("p (c n) -> p c n", c=8)
                                 [:, :, ds(p_off, 8)],
                            start=True, stop=True)
                        for c in range(8):
                            for kt in (1, 2):
                                nc.tensor.matmul(
                                    out=g_ps[:, c * 8:(c + 1) * 8],
                                    lhsT=wTd(d, kt)[:, c * 128:(c + 1) * 128],
                                    rhs=hp[:, (kt - 1) * 8:kt * 8],
                                    start=False, stop=False,
                                    skip_group_check=True)
                        sg = sp.tile([128, 64], dt.float32, tag="s")
                        nc.scalar.activation(out=sg[:, 0:48],
                                             in_=g_ps[:, 0:48],
                                             func=AF.Sigmoid)
                        nc.scalar.activation(out=sg[:, 48:64],
                                             in_=g_ps[:, 48:64],
                                             func=AF.Tanh)
                        # f*c_prev -> scratch (must read c_d before overwrite)
                        nc.vector.tensor_tensor(out=sg[:, 16:32],
                                                in0=sg[:, 16:32],
                                                in1=c_d[:], op=OP.mult)
                        nc.vector.tensor_tensor(out=c_d[:], in0=sg[:, 0:16],
                                                in1=sg[:, 48:64], op=OP.mult)
                        nc.vector.tensor_tensor(out=c_d[:], in0=c_d[:],
                                                in1=sg[:, 16:32], op=OP.add)
                        return sg

                    def scan_step_b(d, s, u, sg):
                        """tanh(c) + h writes for direction d."""
                        h_d = h_f if d == 0 else h_b
                        c_d = c_f if d == 0 else c_b
                        hw_off = (s * 16 + 16 if d == 0 else
                                  (L_eff - 1) * 16 - s * 16)
                        hw = h_pp[d][(u + 1) % 2]
                        nc.scalar.activation(out=sg[:, 48:64], in_=c_d[:],
                                             func=AF.Tanh)
                        nc.vector.tensor_tensor(out=hw[:],
                                                in0=sg[:, 32:48],
                                                in1=sg[:, 48:64], op=OP.mult)
                        # t-indexed history for emission: duplicate the
                        # multiply on the (idle) Pool engine, off the
                        # recurrence critical path.
                        nc.gpsimd.tensor_tensor(out=h_d[:, ds(hw_off, 16)],
                                                in0=sg[:, 32:48],
                                                in1=sg[:, 48:64], op=OP.mult)

                    with tc.For_i(0, 0 if skip_scan else L_eff, U,
                                  staggered_reset=staggered,
                                  hint_engines=hints) as s0:
                        for u in range(U):
                            sg0 = scan_step_a(0, s0 + u, u)
                            sg1 = scan_step_a(1, s0 + u, u)
                            scan_step_b(0, s0 + u, u, sg0)
                            scan_step_b(1, s0 + u, u, sg1)

                    # ---- phase 3: emission ----
                    for blk in range(0 if skip_rest else NEB):
                        t0 = blk * (EB // 8)
                        nt = EB // 8
                        eps = ep.tile([32, EB], dt.float32, tag="e")
                        for kt in range(4):
                            h_d = h_f if kt < 2 else h_b
                            c = kt % 2
                            off = 1 if kt < 2 else 0  # fwd slot t+1, bwd t
                            rhs = h_d.rearrange("p (t x) -> p t x", x=16)[
                                :, t0 + off:t0 + off + nt, c * 8:(c + 1) * 8]
                            nc.tensor.matmul(
                                out=eps[:],
                                lhsT=wlin_sb[:, kt * 32:(kt + 1) * 32],
                                rhs=rhs, start=(kt == 0), stop=(kt == 3))
                        esb = mp.tile([32, EB], dt.float32, tag="m")
                        nc.vector.tensor_copy(out=esb[:], in_=eps[:])
                        nc.sync.dma_start(
                            out=d_out[:, blk * EB:(blk + 1) * EB],
                            in_=esb[:])

                if timing:
                    tl = mp.tile([1, 16], dt.float32, tag="tl")
                    nc.sync.dma_start(out=tl[:], in_=d_out[0:1, 0:16])
                    nc.sync.dma_start(out=d_out_ext[:], in_=tl[:])

    nc.finalize()
    return nc


_CACHE = {}


def _get_nc(L_eff=L, reps=1, timing=False):
    key = (L_eff, reps, timing)
    if key not in _CACHE:
        _CACHE[key] = build_nc(L_eff, reps, timing)
    return _CACHE[key]


_RUNNERS = {}


def _make_runner(nc, n_cores):
    """Persistent jitted executor: the same bass2jax/PJRT path that
    run_bass_kernel_spmd takes under axon, but the jit/shard_map callable is
    built once and cached, so repeat kernel() calls skip the per-call
    retrace + XLA recompile + NEFF device reload (~2s each)."""
    import jax
    import numpy as _np
    from jax.sharding import Mesh, PartitionSpec
    import warnings
    try:
        with warnings.catch_warnings():
            warnings.simplefilter("ignore")
            from jax.experimental.shard_map import shard_map

            def _smap(f, mesh, in_specs, out_specs):
                return shard_map(f, mesh=mesh, in_specs=in_specs,
                                 out_specs=out_specs, check_rep=False)
    except ImportError:
        from jax import shard_map as _sm

        def _smap(f, mesh, in_specs, out_specs):
            return _sm(f, mesh=mesh, in_specs=in_specs,
                       out_specs=out_specs, check_vma=False)
    import concourse.mybir as mybir
    from concourse import bass2jax
    from concourse.bass2jax import _bass_exec_p, install_neuronx_cc_hook

    install_neuronx_cc_hook()
    partition_name = (nc.partition_id_tensor.name
                      if nc.partition_id_tensor else None)
    in_names, out_names, out_avals = [], [], []
    for alloc in nc.m.functions[0].allocations:
        if not isinstance(alloc, mybir.MemoryLocationSet):
            continue
        name = alloc.memorylocations[0].name
        if alloc.kind == "ExternalInput":
            if name != partition_name:
                in_names.append(name)
        elif alloc.kind == "ExternalOutput":
            out_names.append(name)
            out_avals.append(jax.core.ShapedArray(
                tuple(alloc.tensor_shape), mybir.dt.np(alloc.dtype)))
    n_params = len(in_names)
    all_in_names = list(in_names) + list(out_names)
    if partition_name is not None:
        all_in_names.append(partition_name)

    def _body(*args):
        operands = list(args)
        if partition_name is not None:
            operands.append(bass2jax.partition_id_tensor())
        outs = _bass_exec_p.bind(
            *operands,
            out_avals=tuple(out_avals),
            in_names=tuple(all_in_names),
            out_names=tuple(out_names),
            lowering_input_output_aliases=(),
            sim_require_finite=True,
            sim_require_nnan=True,
            nc=nc,
        )
        return tuple(outs)

    devices = jax.devices()[:n_cores]
    mesh = Mesh(_np.asarray(devices), ("core",))
    n_outs = len(out_avals)
    sharded = jax.jit(
        _smap(_body, mesh,
              (PartitionSpec("core"),) * (n_params + n_outs),
              (PartitionSpec("core"),) * n_outs),
        keep_unused=True,
    )

    # Device-resident input cache keyed by content hash: repeat kernel()
    # calls with unchanged weights/tokens skip the ~20MB tunnel re-upload.
    # (Transfer caching only — the device still executes every call.)
    import hashlib
    from jax.sharding import NamedSharding
    sharding = NamedSharding(mesh, PartitionSpec("core"))
    dev_cache = {}

    def _put(name, per_core):
        h = hashlib.blake2b(digest_size=16)
        for a in per_core:
            h.update(_np.ascontiguousarray(a).view(_np.uint8))
        key = (h.hexdigest(), tuple(per_core[0].shape))
        ent = dev_cache.get(name)
        if ent is not None and ent[0] == key:
            return ent[1]
        dev = jax.device_put(_np.concatenate(per_core, axis=0), sharding)
        dev_cache[name] = (key, dev)
        return dev

    def run(in_maps):
        concat_in = [
            _put(name, [_np.asarray(m[name]) for m in in_maps])
            for name in in_names
        ]
        if "zeros" not in dev_cache:
            dev_cache["zeros"] = [
                jax.device_put(
                    _np.zeros((n_cores * a.shape[0], *a.shape[1:]), a.dtype),
                    sharding)
                for a in out_avals
            ]
        out = sharded(*concat_in, *dev_cache["zeros"])
        return [
            {name: _np.asarray(out[i]).reshape(n_cores, *out_avals[i].shape)[c]
             for i, name in enumerate(out_names)}
            for c in range(n_cores)
        ]

    return run


def _run_spmd(nc, in_maps):
    key = id(nc)
    if key not in _RUNNERS:
        _RUNNERS[key] = _make_runner(nc, NCORES)
    return _RUNNERS[key](in_maps)


def _assemble(outs, inputs, L_eff=L):
    """Host CRF broadcast: out[b,l,i,j] = emis[b,l,j] + trans[i,j] + b_lin[j]."""
    import concurrent.futures as cf

    M = (np.asarray(inputs["transition"], np.float32)
         + np.asarray(inputs["b_lin"], np.float32)[None, :])  # [32, 32]
    res = np.empty((B, L_eff, OUT, OUT), np.float32)

    def fill(c):
        emis = outs[c].reshape(32, L_eff, BC).transpose(2, 1, 0)  # [8, L, 32]
        np.add(emis[:, :, None, :], M[None, None, :, :],
               out=res[c * BC:(c + 1) * BC])

    with cf.ThreadPoolExecutor(max_workers=8) as ex:
        list(ex.map(fill, range(NCORES)))
    return res


def kernel(**inputs):
    nc = _get_nc(L, 1, False)
    in_maps = _host_prep(inputs, L)
    try:
        results = _run_spmd(nc, in_maps)
    except Exception:
        # fallback: per-call path through bass_utils (retraces every call)
        from concourse.bass_utils import run_bass_kernel_spmd
        results = run_bass_kernel_spmd(nc, in_maps,
                                       list(range(NCORES))).results
    outs = [results[c]["out"] for c in range(NCORES)]
    return _assemble(outs, inputs, L)


if __name__ == "__main__":
    nc = build_nc(64)
    print("built OK")
